# revision 1
# baseline (speedup 1.0000x reference)
"""AttentionBlock3D (GroupNorm + 8-head self-attention over 16^3 voxels +
out-projection + residual) on 8 TRN2 NeuronCores.

Sharding: one attention head per core (H=8). Every core:
  - loads the full x [64, 4096], computes GroupNorm (stats via bn_stats +
    block-diag matmul group-combine),
  - computes its head's q,k (one [64,16] matmul) and v^T (per-t-block
    matmuls producing the transposed v directly),
  - streams flash-attention-style over the 4096x4096 score matrix:
    scores^T tile = k_blk^T q  (PE, fp16), exp on ScalarE (PSUM->SBUF,
    fused *scale, -6.0 offset; constant offset cancels in softmax),
    out_aug accumulation via PE with v^T augmented by a ones column so the
    softmax denominator falls out of the same matmul,
  - divides, projects through its slice of out_w, writes a partial [64,4096].
Host gathers: out = sum(partials) + out_b + x.
"""
import os
from contextlib import ExitStack

import numpy as np

import concourse.bass as bass
import concourse.tile as tile
from concourse import bacc, mybir
from concourse.bass import ts
from concourse.bass_utils import run_bass_kernel_spmd

C, H, G, D = 64, 8, 8, 8
S = 4096
EPS = 1e-5
SCALE = float(D) ** -0.5
EXP_OFF = -6.0          # constant exp offset; cancels in softmax division

SC = 512                # s-chunk (one PSUM bank of fp32)
NSC = S // SC           # 8
TB = 128                # t-block (partition dim of scores^T tiles)
NTB = S // TB           # 32

# tunables (sweepable via _build(cfg=...))
DEFAULT_CFG = {
    "BT": 3,            # t-blocks per exp batch
    "SC_BUFS": 2,       # scores psum tile double-buffering
    "EXP_BUFS": 3,      # exp sbuf tile buffers
    "DVE_QK_COPY": False,   # do the qk PSUM->SBUF copy on DVE instead of ACT
    "CHUNK_PROLOGUE": False,  # chunk x DMA + xn so stats overlap the load
    "FAKE_EXP_DVE": False,  # perf probe: replace exp with DVE copy (WRONG math)
    "NTB_LIMIT": NTB,       # perf probe: process only this many t-blocks
    "SKIP_FIN": False,      # perf probe: skip per-chunk finalize + output DMA
    "EXP_F32": False,       # exp output (and PV moving operand) in fp32
    "FIN_IN_SC": False,     # allocate finalize PSUM tiles from the scores pool
    "OUT_BUFS": 1,          # out_ps accumulator buffers
    "SPLIT_EXP": False,     # one exp instruction per 512-wide bank
    "V2": False,            # row-tiled QK (3 strips) + 2-strip PV attention loop
}

F32 = mybir.dt.float32
F16 = mybir.dt.float16

_NC_CACHE = None


def _emit(nc, reps=1, cfg=DEFAULT_CFG):
    x = nc.dram_tensor("x", [C, S], F32, kind="ExternalInput").ap()
    gamma = nc.dram_tensor("gamma", [C, 1], F32, kind="ExternalInput").ap()
    beta = nc.dram_tensor("beta", [C, 1], F32, kind="ExternalInput").ap()
    gdiag = nc.dram_tensor("gdiag", [C, C], F32, kind="ExternalInput").ap()
    wqk = nc.dram_tensor("wqk", [C, 2 * D], F32, kind="ExternalInput").ap()
    wv = nc.dram_tensor("wv", [C, D], F32, kind="ExternalInput").ap()
    wo = nc.dram_tensor("wo", [D, C], F32, kind="ExternalInput").ap()
    part = nc.dram_tensor("part", [C, S], F32, kind="ExternalOutput").ap()

    with tile.TileContext(nc) as tc:
        if reps == 1:
            _body(nc, tc, x, gamma, beta, gdiag, wqk, wv, wo, part, cfg)
        else:
            # benchmark variant: repeat the whole kernel body on-device
            with tc.For_i(0, reps, 1, hint_engines=(mybir.EngineType.PE,)):
                _body(nc, tc, x, gamma, beta, gdiag, wqk, wv, wo, part, cfg)


def _body(nc, tc, x, gamma, beta, gdiag, wqk, wv, wo, part, cfg=DEFAULT_CFG):
    BT = cfg["BT"]
    with ExitStack() as ctx:
        const = ctx.enter_context(tc.tile_pool(name="const", bufs=1))
        big = ctx.enter_context(tc.tile_pool(name="big", bufs=1))
        small = ctx.enter_context(tc.tile_pool(name="small", bufs=1))

        # ---- load inputs ----
        x_sb = big.tile([C, S], F32, name="x_sb")
        if cfg["CHUNK_PROLOGUE"]:
            for j in range(NSC):
                nc.sync.dma_start(out=x_sb[:, ts(j, SC)], in_=x[:, ts(j, SC)])
        else:
            nc.sync.dma_start(out=x_sb[:], in_=x)
        gamma_sb = const.tile([C, 1], F32, name="gamma_sb")
        nc.sync.dma_start(out=gamma_sb[:], in_=gamma)
        beta_sb = const.tile([C, 1], F32, name="beta_sb")
        nc.sync.dma_start(out=beta_sb[:], in_=beta)
        gdiag_sb = const.tile([C, C], F32, name="gdiag_sb")
        nc.sync.dma_start(out=gdiag_sb[:], in_=gdiag)
        wqk_sb = const.tile([C, 2 * D], F32, name="wqk_sb")
        nc.sync.dma_start(out=wqk_sb[:], in_=wqk)
        wv_sb = const.tile([C, D], F32, name="wv_sb")
        nc.sync.dma_start(out=wv_sb[:], in_=wv)
        wo_sb = const.tile([D, C], F32, name="wo_sb")
        nc.sync.dma_start(out=wo_sb[:], in_=wo)
        eps_sb = const.tile([C, 1], F32, name="eps_sb")
        nc.vector.memset(eps_sb[:], EPS)
        ones_sb = const.tile([1, D], F32, name="ones_sb")
        nc.vector.memset(ones_sb[:], 1.0)
        zero_sb = const.tile([C, 1], F32, name="zero_sb")
        nc.vector.memset(zero_sb[:], 0.0)
        expoff_sb = const.tile([TB, 1], F32, name="expoff_sb")
        nc.vector.memset(expoff_sb[:], EXP_OFF)

        # ---- GroupNorm stats (per-channel bn_stats, then group combine) ----
        NSUB = S // 512
        stats = small.tile([C, NSUB, 6], F32, name="stats")
        xv = x_sb[:].rearrange("p (n f) -> p n f", f=512)
        for i in range(NSUB):
            nc.vector.bn_stats(out=stats[:, i, :], in_=xv[:, i, :])
        mv = small.tile([C, 2], F32, name="mv")
        nc.vector.bn_aggr(out=mv[:], in_=stats[:])

        # m2 = [mean_c, E[x^2]_c]
        m2 = small.tile([C, 2], F32, name="m2")
        nc.vector.tensor_copy(out=m2[:, 0:1], in_=mv[:, 0:1])
        nc.vector.tensor_mul(out=m2[:, 1:2], in0=mv[:, 0:1], in1=mv[:, 0:1])
        nc.vector.tensor_add(out=m2[:, 1:2], in0=m2[:, 1:2], in1=mv[:, 1:2])

        gst = small.tile([C, 2], F32, name="gst")
        with tc.tile_pool(name="pre_ps", bufs=1, space="PSUM") as pre_ps:
            gst_ps = pre_ps.tile([C, 2], F32, name="gst_ps")
            nc.tensor.matmul(gst_ps[:], lhsT=gdiag_sb[:], rhs=m2[:],
                             start=True, stop=True)
            nc.vector.tensor_copy(out=gst[:], in_=gst_ps[:])

        # var_g = E[x^2]_g - mean_g^2 ; rstd = exp(-0.5*ln(var+eps))
        var = small.tile([C, 1], F32, name="var")
        nc.vector.tensor_mul(out=var[:], in0=gst[:, 0:1], in1=gst[:, 0:1])
        nc.vector.tensor_sub(out=var[:], in0=gst[:, 1:2], in1=var[:])
        rstd = small.tile([C, 1], F32, name="rstd")
        nc.scalar.activation(out=rstd[:], in_=var[:],
                             func=mybir.ActivationFunctionType.Ln,
                             bias=eps_sb[:], scale=1.0)
        nc.scalar.activation(out=rstd[:], in_=rstd[:],
                             func=mybir.ActivationFunctionType.Exp,
                             bias=zero_sb[:], scale=-0.5)
        a_sc = small.tile([C, 1], F32, name="a_sc")
        nc.vector.tensor_mul(out=a_sc[:], in0=rstd[:], in1=gamma_sb[:])
        b_sc = small.tile([C, 1], F32, name="b_sc")
        nc.vector.tensor_mul(out=b_sc[:], in0=gst[:, 0:1], in1=a_sc[:])
        nc.vector.tensor_sub(out=b_sc[:], in0=beta_sb[:], in1=b_sc[:])

        xn_sb = big.tile([C, S], F32, name="xn_sb")
        nc.vector.tensor_scalar(out=xn_sb[:], in0=x_sb[:],
                                scalar1=a_sc[:], scalar2=b_sc[:],
                                op0=mybir.AluOpType.mult,
                                op1=mybir.AluOpType.add)

        # ---- q, k for this head (fp16), one [64,16]x[64,S] matmul ----
        # Engine accesses must start at 32-aligned partitions, so copy the
        # [16,S] PSUM result as one block, then peel k off with a DMA
        # (DMAs may start at any partition).
        qk_sb = big.tile([2 * D, S], F16, name="qk_sb")
        k_sb = big.tile([D, S], F16, name="k_sb")
        with tc.tile_pool(name="qkv_ps", bufs=1, space="PSUM") as qkv_pool:
            qk_ps = qkv_pool.tile([2 * D, S], F32, name="qk_ps")
            for j in range(NSC):
                nc.tensor.matmul(qk_ps[:, ts(j, SC)], lhsT=wqk_sb[:],
                                 rhs=xn_sb[:, ts(j, SC)], start=True, stop=True)
            if cfg["DVE_QK_COPY"]:
                nc.vector.tensor_copy(out=qk_sb[:], in_=qk_ps[:])
            else:
                nc.scalar.copy(out=qk_sb[:], in_=qk_ps[:])
        nc.sync.dma_start(out=k_sb[:], in_=qk_sb[D:2 * D, :])
        q_sb = qk_sb  # rows 0:D are q (base partition 0)
        if cfg["V2"]:
            # zero-fill so the unused rows of each 32-row strip contribute
            # zero terms to the K=32 contraction
            q_rep = big.tile([TB, S], F16, name="q_rep")
            k_rep = big.tile([TB, S], F16, name="k_rep")
            nc.vector.memset(q_rep[:], 0.0)
            nc.vector.memset(k_rep[:], 0.0)
            for r in range(4):
                nc.sync.dma_start(out=q_rep[32 * r:32 * r + D, :],
                                  in_=qk_sb[0:D, :])
                nc.sync.dma_start(out=k_rep[32 * r:32 * r + D, :],
                                  in_=qk_sb[D:2 * D, :])

        # ---- v^T padded to 33 cols: 0:8 = v, 8:32 = 0, 32 = ones ----
        # (the PV matmul then emits the softmax denominator on PSUM
        # partition 32, which is a legal engine-access base)
        MAUG = 33
        vT_sb = big.tile([TB, NTB, MAUG],
                         F32 if cfg["EXP_F32"] else F16, name="vT_sb")
        nc.vector.memset(vT_sb[:], 0.0)
        nc.vector.memset(vT_sb[:, :, MAUG - 1:MAUG], 1.0)
        with tc.tile_pool(name="vt_ps", bufs=1, space="PSUM") as vt_pool:
            vt_ps = vt_pool.tile([TB, NTB, D], F32, name="vt_ps")
            for i in range(NTB):
                nc.tensor.matmul(vt_ps[:, i, :], lhsT=xn_sb[:, ts(i, TB)],
                                 rhs=wv_sb[:], start=True, stop=True)
            nc.scalar.copy(out=vT_sb[:, :, 0:D], in_=vt_ps[:])

        # ---- attention main loop ----
        sc_pool = ctx.enter_context(tc.tile_pool(name="sc_ps", bufs=cfg["SC_BUFS"], space="PSUM"))
        exp_pool = ctx.enter_context(tc.tile_pool(name="exp_sb", bufs=cfg["EXP_BUFS"]))
        outp_pool = ctx.enter_context(tc.tile_pool(name="out_ps", bufs=cfg["OUT_BUFS"], space="PSUM"))
        fin_ps_pool = (None if cfg["FIN_IN_SC"] else
                       ctx.enter_context(tc.tile_pool(name="fin_ps", bufs=1, space="PSUM")))
        fin_sb_pool = ctx.enter_context(tc.tile_pool(name="fin_sb", bufs=2))
        osb_pool = ctx.enter_context(tc.tile_pool(name="o_sb", bufs=2))

        batches = [BT] * (NTB // BT) + ([NTB % BT] if NTB % BT else [])

        if cfg["V2"]:
            _attn_v2(nc, tc, ctx, cfg, q_rep, k_rep, vT_sb, wo_sb, ones_sb,
                     expoff_sb, part)
            return

        ntb_lim = cfg["NTB_LIMIT"]
        use_batches = []
        left = ntb_lim
        for nb in batches:
            if left <= 0:
                break
            use_batches.append(min(nb, left))
            left -= nb
        last_t = sum(use_batches) - 1

        for s in range(NSC):
            out_ps = (outp_pool.tile([MAUG, SC], F32, name="out_ps")
                      if use_batches else None)
            tb0 = 0
            for nb in use_batches:
                scp = sc_pool.tile([TB, BT * SC], F32, name="scp")
                expt = exp_pool.tile([TB, BT * SC],
                                     F32 if cfg["EXP_F32"] else F16, name="expt")
                for j in range(nb):
                    t = tb0 + j
                    nc.tensor.matmul(scp[:, ts(j, SC)],
                                     lhsT=k_sb[:, ts(t, TB)],
                                     rhs=q_sb[0:D, ts(s, SC)],
                                     start=True, stop=True)
                if cfg["FAKE_EXP_DVE"]:
                    nc.vector.tensor_copy(out=expt[:, 0:nb * SC],
                                          in_=scp[:, 0:nb * SC])
                elif cfg["SPLIT_EXP"]:
                    for j in range(nb):
                        nc.scalar.activation(out=expt[:, ts(j, SC)],
                                             in_=scp[:, ts(j, SC)],
                                             func=mybir.ActivationFunctionType.Exp,
                                             bias=expoff_sb[:], scale=SCALE)
                else:
                    nc.scalar.activation(out=expt[:, 0:nb * SC],
                                         in_=scp[:, 0:nb * SC],
                                         func=mybir.ActivationFunctionType.Exp,
                                         bias=expoff_sb[:], scale=SCALE)
                for j in range(nb):
                    t = tb0 + j
                    nc.tensor.matmul(out_ps[:], lhsT=vT_sb[:, t, :],
                                     rhs=expt[:, ts(j, SC)],
                                     start=(t == 0), stop=(t == last_t))
                tb0 += nb

            # finalize: divide by row-sum, project, store
            if cfg["SKIP_FIN"] or out_ps is None:
                continue
            recip = fin_sb_pool.tile([1, SC], F32, name="recip")
            nc.vector.reciprocal(out=recip[:], in_=out_ps[MAUG - 1:MAUG, :])
            fin_pool = sc_pool if cfg["FIN_IN_SC"] else fin_ps_pool
            fin_tag = "scp" if cfg["FIN_IN_SC"] else "fin"
            bcast_ps = fin_pool.tile([D, SC], F32, name="bcast_ps", tag=fin_tag)
            nc.tensor.matmul(bcast_ps[:], lhsT=ones_sb[:], rhs=recip[:],
                             start=True, stop=True)
            bcast_sb = fin_sb_pool.tile([D, SC], F32, name="bcast_sb")
            nc.vector.tensor_copy(out=bcast_sb[:], in_=bcast_ps[:])
            attn_sb = fin_sb_pool.tile([D, SC], F32, name="attn_sb")
            nc.vector.tensor_mul(out=attn_sb[:], in0=out_ps[0:D, :],
                                 in1=bcast_sb[:])
            proj_ps = fin_pool.tile([C, SC], F32, name="proj_ps", tag=fin_tag)
            nc.tensor.matmul(proj_ps[:], lhsT=wo_sb[:], rhs=attn_sb[:],
                             start=True, stop=True)
            o_sb = osb_pool.tile([C, SC], F32, name="o_sb")
            nc.vector.tensor_copy(out=o_sb[:], in_=proj_ps[:])
            nc.sync.dma_start(out=part[:, ts(s, SC)], in_=o_sb[:])


_NC_CACHE_REPS = {}


def _build(reps=1, cfg=None):
    global _NC_CACHE_REPS
    full = dict(DEFAULT_CFG)
    if cfg:
        full.update(cfg)
    key = (reps, tuple(sorted(full.items())))
    if key in _NC_CACHE_REPS:
        return _NC_CACHE_REPS[key]
    nc = bacc.Bacc("TRN2", target_bir_lowering=False, debug=False)
    _emit(nc, reps=reps, cfg=full)
    nc.compile()
    _NC_CACHE_REPS[key] = nc
    return nc


def _host_inputs(inputs):
    x = np.ascontiguousarray(np.asarray(inputs["x"], dtype=np.float32))
    gn_w = np.asarray(inputs["gn_weight"], dtype=np.float32).reshape(C, 1)
    gn_b = np.asarray(inputs["gn_bias"], dtype=np.float32).reshape(C, 1)
    qkv_w = np.asarray(inputs["qkv_w"], dtype=np.float32)
    out_w = np.asarray(inputs["out_w"], dtype=np.float32)

    x2 = np.ascontiguousarray(x.reshape(C, S))
    gd = np.kron(np.eye(G, dtype=np.float32),
                 np.full((C // G, C // G), float(G) / C, dtype=np.float32))
    gd = np.ascontiguousarray(gd)

    in_maps = []
    for h in range(H):
        rq = np.arange(h * D, (h + 1) * D)
        wqk_h = np.ascontiguousarray(
            qkv_w[np.concatenate([rq, C + rq])].T)          # [64, 16]
        wv_h = np.ascontiguousarray(qkv_w[2 * C + rq].T)    # [64, 8]
        wo_h = np.ascontiguousarray(out_w[:, rq].T)         # [8, 64]
        in_maps.append({
            "x": x2, "gamma": gn_w, "beta": gn_b, "gdiag": gd,
            "wqk": wqk_h, "wv": wv_h, "wo": wo_h,
        })
    return in_maps, x2


def kernel(**inputs):
    x = np.asarray(inputs["x"])
    out_b = np.asarray(inputs["out_b"], dtype=np.float32)
    in_maps, x2 = _host_inputs(inputs)

    nc = _build()
    trace = bool(int(os.environ.get("KERNEL_TRACE", "0")))
    res = run_bass_kernel_spmd(nc, in_maps, core_ids=list(range(H)),
                               trace=trace)
    if trace:
        kernel.last_results = res

    acc = np.zeros((C, S), dtype=np.float32)
    for r in res.results:
        acc += r["part"]
    out = acc + out_b[:, None] + x2
    return out.reshape(x.shape).astype(np.float32)


def _attn_v2(nc, tc, ctx, cfg, q_rep, k_rep, vT_sb, wo_sb, ones_sb,
             expoff_sb, part):
    """Attention v2: 3-strip row-tiled QK, software-pipelined emission.

    Emission order per unit u: QK(u) -> PV(u-1) -> deferred finalize -> exp(u),
    so the PE never queues behind an exp it doesn't depend on, and the
    per-chunk finalize matmuls sit behind the next chunk's first QK batch.
    PSUM: scp 2x3 banks + out_ps 2x1 = 8; finalize tiles borrow scp slots.
    """
    BT3 = 3
    MAUG = 33
    sc_pool = ctx.enter_context(
        tc.tile_pool(name="sc2_ps", bufs=2, space="PSUM"))
    exp_pool = ctx.enter_context(tc.tile_pool(name="exp2_sb", bufs=3))
    outp_pool = ctx.enter_context(
        tc.tile_pool(name="out2_ps", bufs=2, space="PSUM"))
    fin_sb_pool = ctx.enter_context(tc.tile_pool(name="fin2_sb", bufs=2))
    osb_pool = ctx.enter_context(tc.tile_pool(name="o2_sb", bufs=2))

    batches = [BT3] * (NTB // BT3) + ([NTB % BT3] if NTB % BT3 else [])

    pending = []          # deferred emission closures, FIFO

    def flush():
        n = len(pending)
        for _ in range(n):
            pending.pop(0)()

    state = {"out_ps": None}

    def emit_fin(out_ps, s):
        def fin():
            recip = fin_sb_pool.tile([1, SC], F32, name="recip2")
            nc.vector.reciprocal(out=recip[:], in_=out_ps[MAUG - 1:MAUG, :])
            bcast_ps = sc_pool.tile([D, SC], F32, name="bcast2", tag="scp2")
            nc.tensor.matmul(bcast_ps[:], lhsT=ones_sb[:], rhs=recip[:],
                             start=True, stop=True)
            bcast_sb = fin_sb_pool.tile([D, SC], F32, name="bcast2_sb")
            nc.vector.tensor_copy(out=bcast_sb[:], in_=bcast_ps[:])
            attn_sb = fin_sb_pool.tile([D, SC], F32, name="attn2")
            nc.vector.tensor_mul(out=attn_sb[:], in0=out_ps[0:D, :],
                                 in1=bcast_sb[:])
            proj_ps = sc_pool.tile([C, SC], F32, name="proj2", tag="scp2")
            nc.tensor.matmul(proj_ps[:], lhsT=wo_sb[:], rhs=attn_sb[:],
                             start=True, stop=True)
            o_sb = osb_pool.tile([C, SC], F32, name="o2")
            nc.vector.tensor_copy(out=o_sb[:], in_=proj_ps[:])
            nc.sync.dma_start(out=part[:, ts(s, SC)], in_=o_sb[:])
        return fin

    for s in range(NSC):
        tb0 = 0
        for bi, nb in enumerate(batches):
            scp = sc_pool.tile([TB, BT3 * SC], F32, name="scp2")
            expt = exp_pool.tile([TB, BT3 * SC], F16, name="expt2")
            for r in range(nb):
                t = tb0 + r
                nc.tensor.matmul(scp[:, ts(r, SC)],
                                 lhsT=k_rep[32 * r:32 * r + 32, ts(t, TB)],
                                 rhs=q_rep[32 * r:32 * r + 32, ts(s, SC)],
                                 start=True, stop=True,
                                 tile_position=(32 * r, 0))
            # previous unit's PV (and any deferred finalize) go behind this QK
            flush()
            nc.scalar.activation(out=expt[:, 0:nb * SC],
                                 in_=scp[:, 0:nb * SC],
                                 func=mybir.ActivationFunctionType.Exp,
                                 bias=expoff_sb[:], scale=SCALE)

            def emit_pv(s=s, bi=bi, nb=nb, tb0=tb0, expt=expt):
                if bi == 0:
                    state["out_ps"] = outp_pool.tile([MAUG, SC], F32,
                                                     name="out2_ps_t")
                out_ps = state["out_ps"]
                for j in range(nb):
                    t = tb0 + j
                    nc.tensor.matmul(out_ps[:], lhsT=vT_sb[:, t, :],
                                     rhs=expt[:, ts(j, SC)],
                                     start=(t == 0), stop=(t == NTB - 1))
                if t == NTB - 1:
                    pending.append(emit_fin(out_ps, s))
            pending.append(emit_pv)
            tb0 += nb
    flush()



# revision 5
# speedup vs baseline: 1.4616x; 1.4616x over previous
"""AttentionBlock3D (GroupNorm + 8-head self-attention over 16^3 voxels +
out-projection + residual) on 8 TRN2 NeuronCores — one head per core.

Per-core pipeline:
  - chunked x load + GroupNorm stats (bn_stats + block-diag matmul combine),
    xn = a*x + b stored fp16,
  - q,k via one [64,16] matmul per s-chunk; q/k replicated 16x along
    partitions (fp16) so QK^T runs as 4 concurrent 32-row PE tiles
    (tile_position row tiling; the 4x replication factor folds into the
    softmax scale),
  - v^T per t-block with an appended ones column (emits the softmax
    denominator from the same matmul),
  - streaming attention: scores^T tiles on PE -> exp split across TWO
    engines: ScalarE exact exp (fp16 out) and VectorE fast-exp
    (t = score*A + B -> uint16 round/saturate -> bitcast fp16; negatives
    saturate to 0 = prob 0), -> PV accumulation as 4 concurrent 32-col
    PE tiles into one PSUM bank (zero-matmul clears has_written),
  - per chunk: copy [128,512] PSUM->SBUF (alternating ScalarE/VectorE),
    DMA out. No on-device division or out-projection.
Host gathers: num_h = sum of 4 col-group rows, attn = num/den,
out = sum_h out_w_h @ attn_h + out_b + x.
"""
import math
import os
from contextlib import ExitStack

import numpy as np

import concourse.bass as bass
import concourse.tile as tile
from concourse import bacc, mybir
from concourse.bass import ts
from concourse.bass_utils import run_bass_kernel_spmd

C, H, G, D = 64, 8, 8, 8
S = 4096
EPS = 1e-5
SCALE = float(D) ** -0.5

SC = 512                # s-chunk (one PSUM bank of fp32)
NSC = S // SC           # 8
TB = 128                # t-block (partition dim of scores^T tiles)
NTB = S // TB           # 32

K2 = -0.5                               # softmax offset, log2 domain
OFF_LN = K2 * math.log(2.0)             # same offset, natural log (ScalarE)
REP = 4                                 # q/k replication factor in each strip
A4 = (SCALE / REP) * math.log2(math.e) * 1024.0   # fast-exp multiplier
B_MAGIC = (K2 + 15.0) * 1024.0 - 61.0             # fast-exp bias + mantissa corr

F32 = mybir.dt.float32
F16 = mybir.dt.float16
U16 = mybir.dt.uint16

# per chunk: N_ACT units of BT_A t-blocks on ScalarE + the rest on VectorE
DEFAULT_CFG = {
    "N_ACT": 9,         # ScalarE exp units per chunk
    "BT_A": 2,          # t-blocks per ScalarE unit (PSUM banks per tile)
    "ACT_SC_BUFS": 2,
    "DVE_SC_BUFS": 2,
    "OUT_BUFS": 2,
    "ACT_EXP_BUFS": 3,
    "DVE_EXP_BUFS": 3,
}


def _interleave(n_a, n_d):
    """Evenly merge n_a 'A' and n_d 'D' into one sequence (Bresenham)."""
    seq = []
    ia = id_ = 0
    for i in range(n_a + n_d):
        # pick the stream that is most behind its target rate
        if ia * (n_a + n_d) <= i * n_a and ia < n_a:
            seq.append("A")
            ia += 1
        elif id_ < n_d:
            seq.append("D")
            id_ += 1
        else:
            seq.append("A")
            ia += 1
    return seq


def _emit(nc, cfg=DEFAULT_CFG):
    x = nc.dram_tensor("x", [C, S], F32, kind="ExternalInput").ap()
    gamma = nc.dram_tensor("gamma", [C, 1], F32, kind="ExternalInput").ap()
    beta = nc.dram_tensor("beta", [C, 1], F32, kind="ExternalInput").ap()
    gdiag = nc.dram_tensor("gdiag", [C, C], F32, kind="ExternalInput").ap()
    wqk = nc.dram_tensor("wqk", [C, 2 * D], F16, kind="ExternalInput").ap()
    wv = nc.dram_tensor("wv", [C, D], F16, kind="ExternalInput").ap()
    part = nc.dram_tensor("part", [TB, S], F32, kind="ExternalOutput").ap()

    with tile.TileContext(nc) as tc:
        _body(nc, tc, x, gamma, beta, gdiag, wqk, wv, part, cfg)


def _body(nc, tc, x, gamma, beta, gdiag, wqk, wv, part, cfg):
    n_act = cfg["N_ACT"]
    bt_a = cfg["BT_A"]
    n_dve = NTB - n_act * bt_a
    units = _interleave(n_act, n_dve)

    with ExitStack() as ctx:
        const = ctx.enter_context(tc.tile_pool(name="const", bufs=1))
        big = ctx.enter_context(tc.tile_pool(name="big", bufs=1))
        small = ctx.enter_context(tc.tile_pool(name="small", bufs=1))

        # ---- constants ----
        gamma_sb = const.tile([C, 1], F32, name="gamma_sb")
        nc.sync.dma_start(out=gamma_sb[:], in_=gamma)
        beta_sb = const.tile([C, 1], F32, name="beta_sb")
        nc.sync.dma_start(out=beta_sb[:], in_=beta)
        gdiag_sb = const.tile([C, C], F32, name="gdiag_sb")
        nc.sync.dma_start(out=gdiag_sb[:], in_=gdiag)
        wqk_sb = const.tile([C, 2 * D], F16, name="wqk_sb")
        nc.sync.dma_start(out=wqk_sb[:], in_=wqk)
        wv_sb = const.tile([C, D], F16, name="wv_sb")
        nc.sync.dma_start(out=wv_sb[:], in_=wv)
        eps_sb = const.tile([C, 1], F32, name="eps_sb")
        nc.vector.memset(eps_sb[:], EPS)
        zero_sb = const.tile([C, 1], F32, name="zero_sb")
        nc.vector.memset(zero_sb[:], 0.0)
        expoff_sb = const.tile([TB, 1], F32, name="expoff_sb")
        nc.vector.memset(expoff_sb[:], OFF_LN)
        zeros128 = const.tile([TB, TB], F16, name="zeros128")
        nc.vector.memset(zeros128[:], 0.0)

        # ---- x load + GroupNorm stats, chunked ----
        x_sb = big.tile([C, S], F32, name="x_sb")
        stats = small.tile([C, NSC, 6], F32, name="stats")
        xv = x_sb[:].rearrange("p (n f) -> p n f", f=SC)
        for j in range(NSC):
            nc.sync.dma_start(out=x_sb[:, ts(j, SC)], in_=x[:, ts(j, SC)])
            nc.vector.bn_stats(out=stats[:, j, :], in_=xv[:, j, :])
        mv = small.tile([C, 2], F32, name="mv")
        nc.vector.bn_aggr(out=mv[:], in_=stats[:])

        # m2 = [mean_c, E[x^2]_c]
        m2 = small.tile([C, 2], F32, name="m2")
        nc.vector.tensor_copy(out=m2[:, 0:1], in_=mv[:, 0:1])
        nc.vector.tensor_mul(out=m2[:, 1:2], in0=mv[:, 0:1], in1=mv[:, 0:1])
        nc.vector.tensor_add(out=m2[:, 1:2], in0=m2[:, 1:2], in1=mv[:, 1:2])

        gst = small.tile([C, 2], F32, name="gst")
        with tc.tile_pool(name="pre_ps", bufs=1, space="PSUM") as pre_ps:
            gst_ps = pre_ps.tile([C, 2], F32, name="gst_ps")
            nc.tensor.matmul(gst_ps[:], lhsT=gdiag_sb[:], rhs=m2[:],
                             start=True, stop=True)
            nc.vector.tensor_copy(out=gst[:], in_=gst_ps[:])

        # var_g = E[x^2]_g - mean_g^2 ; rstd = exp(-0.5*ln(var+eps))
        var = small.tile([C, 1], F32, name="var")
        nc.vector.tensor_mul(out=var[:], in0=gst[:, 0:1], in1=gst[:, 0:1])
        nc.vector.tensor_sub(out=var[:], in0=gst[:, 1:2], in1=var[:])
        rstd = small.tile([C, 1], F32, name="rstd")
        nc.scalar.activation(out=rstd[:], in_=var[:],
                             func=mybir.ActivationFunctionType.Ln,
                             bias=eps_sb[:], scale=1.0)
        nc.scalar.activation(out=rstd[:], in_=rstd[:],
                             func=mybir.ActivationFunctionType.Exp,
                             bias=zero_sb[:], scale=-0.5)
        a_sc = small.tile([C, 1], F32, name="a_sc")
        nc.vector.tensor_mul(out=a_sc[:], in0=rstd[:], in1=gamma_sb[:])
        b_sc = small.tile([C, 1], F32, name="b_sc")
        nc.vector.tensor_mul(out=b_sc[:], in0=gst[:, 0:1], in1=a_sc[:])
        nc.vector.tensor_sub(out=b_sc[:], in0=beta_sb[:], in1=b_sc[:])

        # ---- xn (fp16), chunked ----
        xn_sb = big.tile([C, S], F16, name="xn_sb")
        for j in range(NSC):
            nc.vector.tensor_scalar(out=xn_sb[:, ts(j, SC)],
                                    in0=x_sb[:, ts(j, SC)],
                                    scalar1=a_sc[:], scalar2=b_sc[:],
                                    op0=mybir.AluOpType.mult,
                                    op1=mybir.AluOpType.add)

        # ---- q,k per chunk -> qk_sb fp16; v^T per t-block ----
        qk_sb = big.tile([2 * D, S], F16, name="qk_sb")
        vT_sb = big.tile([TB, NTB, D + 1], F16, name="vT_sb")
        nc.vector.memset(vT_sb[:], 1.0)   # ones col; 0:D overwritten below
        with tc.tile_pool(name="qk_ps", bufs=2, space="PSUM") as qk_pool, \
             tc.tile_pool(name="vt_ps", bufs=2, space="PSUM") as vt_pool:
            for j in range(NSC):
                qk_ps = qk_pool.tile([2 * D, SC], F32, name="qk_ps")
                nc.tensor.matmul(qk_ps[:], lhsT=wqk_sb[:],
                                 rhs=xn_sb[:, ts(j, SC)], start=True, stop=True)
                nc.scalar.copy(out=qk_sb[:, ts(j, SC)], in_=qk_ps[:])
                vt_ps = vt_pool.tile([TB, 4, D], F32, name="vt_ps")
                for i in range(4):
                    t = 4 * j + i
                    nc.tensor.matmul(vt_ps[:, i, :],
                                     lhsT=xn_sb[:, ts(t, TB)],
                                     rhs=wv_sb[:], start=True, stop=True)
                nc.vector.tensor_copy(
                    out=vT_sb[:, 4 * j:4 * (j + 1), 0:D], in_=vt_ps[:])

        # ---- replicate q,k 16x along partitions (fp16) ----
        q_rep = big.tile([TB, S], F16, name="q_rep")
        k_rep = big.tile([TB, S], F16, name="k_rep")
        for r in range(TB // D):
            nc.sync.dma_start(out=q_rep[D * r:D * (r + 1), :],
                              in_=qk_sb[0:D, :])
            nc.sync.dma_start(out=k_rep[D * r:D * (r + 1), :],
                              in_=qk_sb[D:2 * D, :])

        # ---- attention main loop ----
        asc_pool = ctx.enter_context(tc.tile_pool(
            name="asc_ps", bufs=cfg["ACT_SC_BUFS"], space="PSUM"))
        dsc_pool = ctx.enter_context(tc.tile_pool(
            name="dsc_ps", bufs=cfg["DVE_SC_BUFS"], space="PSUM"))
        outp_pool = ctx.enter_context(tc.tile_pool(
            name="out_ps", bufs=cfg["OUT_BUFS"], space="PSUM"))
        aexp_pool = ctx.enter_context(tc.tile_pool(
            name="aexp_sb", bufs=cfg["ACT_EXP_BUFS"]))
        dexp_pool = ctx.enter_context(tc.tile_pool(
            name="dexp_sb", bufs=cfg["DVE_EXP_BUFS"]))
        osb_pool = ctx.enter_context(tc.tile_pool(name="o_sb", bufs=2))

        pending = []

        def flush():
            n = len(pending)
            for _ in range(n):
                pending.pop(0)()

        qk_ctr = 0
        state = {"out_ps": None}

        def emit_qk(scp, col, t, s):
            nonlocal qk_ctr
            r = qk_ctr % 4
            qk_ctr += 1
            nc.tensor.matmul(scp[:, ts(col, SC)] if col is not None else scp[:],
                             lhsT=k_rep[32 * r:32 * r + 32, ts(t, TB)],
                             rhs=q_rep[32 * r:32 * r + 32, ts(s, SC)],
                             start=True, stop=True,
                             tile_position=(32 * r, 0))

        def emit_clear(s):
            def clear():
                state["out_ps"] = outp_pool.tile([TB, SC], F32, name="out_ps_t")
                nc.tensor.matmul(state["out_ps"][:], lhsT=zeros128[:],
                                 rhs=q_rep[:, 0:SC], start=True, stop=False)
            return clear

        def emit_pv(expt, tbs):
            out_ps = state["out_ps"]

            def pv():
                for j, t in enumerate(tbs):
                    c = t % 4
                    nc.tensor.matmul(out_ps[32 * c:32 * c + D + 1, :],
                                     lhsT=vT_sb[:, t, :],
                                     rhs=expt[:, ts(j, SC)],
                                     start=False, stop=(t >= NTB - 4),
                                     tile_position=(0, 32 * c))
            return pv

        def emit_fin(s):
            out_ps = state["out_ps"]

            def fin():
                o_sb = osb_pool.tile([TB, SC], F32, name="o_sb")
                if s % 2 == 0:
                    nc.scalar.copy(out=o_sb[:], in_=out_ps[:])
                else:
                    nc.vector.tensor_copy(out=o_sb[:], in_=out_ps[:])
                nc.sync.dma_start(out=part[:, ts(s, SC)], in_=o_sb[:])
            return fin

        for s in range(NSC):
            pending.append(emit_clear(s))
            t0 = 0
            for kind in units:
                nb = bt_a if kind == "A" else 1
                tbs = list(range(t0, t0 + nb))
                t0 += nb
                if kind == "A":
                    scp = asc_pool.tile([TB, bt_a * SC], F32, name="ascp")
                    expt = aexp_pool.tile([TB, bt_a * SC], F16, name="aexpt")
                else:
                    scp = dsc_pool.tile([TB, SC], F32, name="dscp")
                    expt = dexp_pool.tile([TB, SC], F16, name="dexpt")
                for j, t in enumerate(tbs):
                    emit_qk(scp, j if nb > 1 else None, t, s)
                flush()
                if kind == "A":
                    nc.scalar.activation(out=expt[:, 0:nb * SC],
                                         in_=scp[:, 0:nb * SC],
                                         func=mybir.ActivationFunctionType.Exp,
                                         bias=expoff_sb[:], scale=SCALE / REP)
                else:
                    nc.vector.tensor_scalar(out=expt[:].bitcast(U16),
                                            in0=scp[:],
                                            scalar1=A4, scalar2=B_MAGIC,
                                            op0=mybir.AluOpType.mult,
                                            op1=mybir.AluOpType.add)
                pending.append(emit_pv(expt, tbs))
            pending.append(emit_fin(s))
        flush()


_NC_CACHE = {}


def _build(cfg=None):
    full = dict(DEFAULT_CFG)
    if cfg:
        full.update(cfg)
    key = tuple(sorted(full.items()))
    if key in _NC_CACHE:
        return _NC_CACHE[key]
    nc = bacc.Bacc("TRN2", target_bir_lowering=False, debug=False)
    _emit(nc, cfg=full)
    nc.compile()
    _NC_CACHE[key] = nc
    return nc


def kernel(**inputs):
    x = np.asarray(inputs["x"])
    out_b = np.asarray(inputs["out_b"], dtype=np.float64)
    out_w = np.asarray(inputs["out_w"], dtype=np.float64)
    gn_w = np.asarray(inputs["gn_weight"], dtype=np.float32).reshape(C, 1)
    gn_b = np.asarray(inputs["gn_bias"], dtype=np.float32).reshape(C, 1)
    qkv_w = np.asarray(inputs["qkv_w"], dtype=np.float32)

    x2 = np.ascontiguousarray(np.asarray(x, dtype=np.float32).reshape(C, S))
    gd = np.kron(np.eye(G, dtype=np.float32),
                 np.full((C // G, C // G), float(G) / C, dtype=np.float32))
    gd = np.ascontiguousarray(gd)

    in_maps = []
    for h in range(H):
        rq = np.arange(h * D, (h + 1) * D)
        wqk_h = np.ascontiguousarray(
            qkv_w[np.concatenate([rq, C + rq])].T.astype(np.float16))
        wv_h = np.ascontiguousarray(qkv_w[2 * C + rq].T.astype(np.float16))
        in_maps.append({
            "x": x2, "gamma": gn_w, "beta": gn_b, "gdiag": gd,
            "wqk": wqk_h, "wv": wv_h,
        })

    nc = _build()
    trace = bool(int(os.environ.get("KERNEL_TRACE", "0")))
    res = run_bass_kernel_spmd(nc, in_maps, core_ids=list(range(H)),
                               trace=trace)
    if trace:
        kernel.last_results = res

    acc = np.zeros((C, S), dtype=np.float64)
    for h, r in enumerate(res.results):
        p = np.asarray(r["part"], dtype=np.float64)
        num = p.reshape(4, 32, S)[:, 0:D + 1, :].sum(axis=0)
        attn = num[0:D] / num[D:D + 1]
        acc += out_w[:, h * D:(h + 1) * D] @ attn
    out = acc + out_b[:, None] + x2.astype(np.float64)
    return out.reshape(x.shape).astype(np.float32)


# revision 7
# speedup vs baseline: 2.3188x; 1.5864x over previous
"""AttentionBlock3D (GroupNorm + 8-head self-attention over 16^3 voxels +
out-projection + residual) on 8 TRN2 NeuronCores — one head per core.

Per-core pipeline:
  - x load (2 halves) + GroupNorm stats (bn_stats + block-diag matmul
    combine), xn = a*x + b stored fp16,
  - q,k via one [64,16] matmul per s-chunk; q/k replicated 16x along
    partitions (fp16, log-depth DMA doubling chains on two queues) so QK^T
    runs as 4 concurrent 32-row PE tiles (tile_position row tiling; the 4x
    replication factor folds into the softmax scale),
  - v^T per t-block with an appended ones column (emits the softmax
    denominator from the same matmul),
  - streaming attention in PACKS of 4 t-blocks (one 2-bank ScalarE tile +
    two 1-bank VectorE tiles): 4-MM QK bursts (row tiles 0..3 concurrent),
    exp split across both engines — ScalarE exact exp (fp16 out) and
    VectorE fast-exp (t = score*A + B -> uint16 round/saturate -> bitcast
    fp16; negatives saturate to 0 = prob 0) — then 4-MM PV bursts
    (col tiles 0..3 concurrent) accumulating into one PSUM bank
    (a zeros-matmul clears has_written each chunk),
  - per chunk: VectorE copies [128,512] PSUM->SBUF, DMA out. No on-device
    division or out-projection.
Host gathers: num_h = sum of 4 col-group rows, attn = num/den,
out = sum_h out_w_h @ attn_h + out_b + x.
"""
import math
import os
from contextlib import ExitStack

import numpy as np

import concourse.bass as bass
import concourse.tile as tile
from concourse import bacc, mybir
from concourse.bass import ts
from concourse.bass_utils import run_bass_kernel_spmd

C, H, G, D = 64, 8, 8, 8
S = 4096
EPS = 1e-5
SCALE = float(D) ** -0.5

SC = 512                # s-chunk (one PSUM bank of fp32)
NSC = S // SC           # 8
TB = 128                # t-block (partition dim of scores^T tiles)
NTB = S // TB           # 32

K2 = -0.5                               # softmax offset, log2 domain
OFF_LN = K2 * math.log(2.0)             # same offset, natural log (ScalarE)
REP = 4                                 # q/k replication factor per strip
A4 = (SCALE / REP) * math.log2(math.e) * 1024.0   # fast-exp multiplier
B_MAGIC = (K2 + 15.0) * 1024.0 - 61.0             # fast-exp bias + mantissa corr

F32 = mybir.dt.float32
F16 = mybir.dt.float16
U16 = mybir.dt.uint16

# packed const layout (fp32 cols): 0 gamma | 1 beta | 2:66 gdiag |
# 66:74 wqk (fp16 pairs) | 74:78 wv (fp16 pairs)
CP_W = 78

DEFAULT_CFG = {
    "ACT_SC_BUFS": 2,
    "DVE_SC_BUFS": 3,
    "ACT_EXP_BUFS": 3,
    "DVE_EXP_BUFS": 3,
}


def _emit(nc, cfg=DEFAULT_CFG):
    x = nc.dram_tensor("x", [C, S], F32, kind="ExternalInput").ap()
    cpack = nc.dram_tensor("cpack", [C, CP_W], F32, kind="ExternalInput").ap()
    part = nc.dram_tensor("part", [TB, S], F32, kind="ExternalOutput").ap()
    with tile.TileContext(nc) as tc:
        _body(nc, tc, x, cpack, part, cfg)


def _body(nc, tc, x, cpack, part, cfg):
    with ExitStack() as ctx:
        const = ctx.enter_context(tc.tile_pool(name="const", bufs=1))
        big = ctx.enter_context(tc.tile_pool(name="big", bufs=1))
        small = ctx.enter_context(tc.tile_pool(name="small", bufs=1))

        # ---- x halves + packed consts ----
        x_sb = big.tile([C, S], F32, name="x_sb")
        nc.sync.dma_start(out=x_sb[:, 0:S // 2], in_=x[:, 0:S // 2])
        nc.sync.dma_start(out=x_sb[:, S // 2:S], in_=x[:, S // 2:S])
        cp_sb = const.tile([C, CP_W], F32, name="cp_sb")
        nc.sync.dma_start(out=cp_sb[:], in_=cpack)
        gamma_sb = cp_sb[:, 0:1]
        beta_sb = cp_sb[:, 1:2]
        gdiag_sb = cp_sb[:, 2:66]
        wqk_sb = cp_sb[:, 66:74].bitcast(F16)     # [64, 16] fp16
        wv_sb = cp_sb[:, 74:78].bitcast(F16)      # [64, 8] fp16

        eps_sb = const.tile([C, 1], F32, name="eps_sb")
        nc.vector.memset(eps_sb[:], EPS)
        zero_sb = const.tile([C, 1], F32, name="zero_sb")
        nc.vector.memset(zero_sb[:], 0.0)
        expoff_sb = const.tile([TB, 1], F32, name="expoff_sb")
        nc.vector.memset(expoff_sb[:], OFF_LN)
        zeros128 = const.tile([TB, TB], F16, name="zeros128")
        nc.vector.memset(zeros128[:], 0.0)

        # ---- GroupNorm stats (per half, 4 bn_stats each) ----
        stats = small.tile([C, NSC, 6], F32, name="stats")
        xv = x_sb[:].rearrange("p (n f) -> p n f", f=SC)
        for j in range(NSC):
            nc.vector.bn_stats(out=stats[:, j, :], in_=xv[:, j, :])
        mv = small.tile([C, 2], F32, name="mv")
        nc.vector.bn_aggr(out=mv[:], in_=stats[:])

        m2 = small.tile([C, 2], F32, name="m2")
        nc.vector.tensor_copy(out=m2[:, 0:1], in_=mv[:, 0:1])
        nc.vector.tensor_mul(out=m2[:, 1:2], in0=mv[:, 0:1], in1=mv[:, 0:1])
        nc.vector.tensor_add(out=m2[:, 1:2], in0=m2[:, 1:2], in1=mv[:, 1:2])

        gst = small.tile([C, 2], F32, name="gst")
        with tc.tile_pool(name="pre_ps", bufs=1, space="PSUM") as pre_ps:
            gst_ps = pre_ps.tile([C, 2], F32, name="gst_ps")
            nc.tensor.matmul(gst_ps[:], lhsT=gdiag_sb, rhs=m2[:],
                             start=True, stop=True)
            nc.vector.tensor_copy(out=gst[:], in_=gst_ps[:])

        var = small.tile([C, 1], F32, name="var")
        nc.vector.tensor_mul(out=var[:], in0=gst[:, 0:1], in1=gst[:, 0:1])
        nc.vector.tensor_sub(out=var[:], in0=gst[:, 1:2], in1=var[:])
        rstd = small.tile([C, 1], F32, name="rstd")
        nc.scalar.activation(out=rstd[:], in_=var[:],
                             func=mybir.ActivationFunctionType.Ln,
                             bias=eps_sb[:], scale=1.0)
        nc.scalar.activation(out=rstd[:], in_=rstd[:],
                             func=mybir.ActivationFunctionType.Exp,
                             bias=zero_sb[:], scale=-0.5)
        a_sc = small.tile([C, 1], F32, name="a_sc")
        nc.vector.tensor_mul(out=a_sc[:], in0=rstd[:], in1=gamma_sb)
        b_sc = small.tile([C, 1], F32, name="b_sc")
        nc.vector.tensor_mul(out=b_sc[:], in0=gst[:, 0:1], in1=a_sc[:])
        nc.vector.tensor_sub(out=b_sc[:], in0=beta_sb, in1=b_sc[:])

        # ---- xn (fp16) + q,k + v^T, chunked ----
        xn_sb = big.tile([C, S], F16, name="xn_sb")
        qk_sb = big.tile([2 * D, S], F16, name="qk_sb")
        vT_sb = big.tile([TB, NTB, D + 1], F16, name="vT_sb")
        nc.vector.memset(vT_sb[:], 1.0)   # ones col; 0:D overwritten below
        with tc.tile_pool(name="qk_ps", bufs=2, space="PSUM") as qk_pool, \
             tc.tile_pool(name="vt_ps", bufs=2, space="PSUM") as vt_pool:
            for j in range(NSC):
                nc.vector.tensor_scalar(out=xn_sb[:, ts(j, SC)],
                                        in0=x_sb[:, ts(j, SC)],
                                        scalar1=a_sc[:], scalar2=b_sc[:],
                                        op0=mybir.AluOpType.mult,
                                        op1=mybir.AluOpType.add)
                qk_ps = qk_pool.tile([2 * D, SC], F32, name="qk_ps")
                nc.tensor.matmul(qk_ps[:], lhsT=wqk_sb,
                                 rhs=xn_sb[:, ts(j, SC)], start=True, stop=True)
                if j % 2 == 0:
                    nc.scalar.copy(out=qk_sb[:, ts(j, SC)], in_=qk_ps[:])
                else:
                    nc.vector.tensor_copy(out=qk_sb[:, ts(j, SC)], in_=qk_ps[:])
                vt_ps = vt_pool.tile([TB, 4, D], F32, name="vt_ps")
                for i in range(4):
                    t = 4 * j + i
                    nc.tensor.matmul(vt_ps[:, i, :],
                                     lhsT=xn_sb[:, ts(t, TB)],
                                     rhs=wv_sb, start=True, stop=True)
                if j % 2 == 0:
                    nc.vector.tensor_copy(
                        out=vT_sb[:, 4 * j:4 * (j + 1), 0:D], in_=vt_ps[:])
                else:
                    nc.scalar.copy(
                        out=vT_sb[:, 4 * j:4 * (j + 1), 0:D], in_=vt_ps[:])

        # ---- replicate q,k 16x along partitions: doubling chains ----
        q_rep = big.tile([TB, S], F16, name="q_rep")
        k_rep = big.tile([TB, S], F16, name="k_rep")
        nc.scalar.dma_start(out=q_rep[0:D, :], in_=qk_sb[0:D, :])
        nc.sync.dma_start(out=k_rep[0:D, :], in_=qk_sb[D:2 * D, :])
        w = D
        while w < TB:
            nc.scalar.dma_start(out=q_rep[w:2 * w, :], in_=q_rep[0:w, :])
            nc.sync.dma_start(out=k_rep[w:2 * w, :], in_=k_rep[0:w, :])
            w *= 2

        # ---- attention main loop ----
        asc_pool = ctx.enter_context(tc.tile_pool(
            name="asc_ps", bufs=cfg["ACT_SC_BUFS"], space="PSUM"))
        dsc_pool = ctx.enter_context(tc.tile_pool(
            name="dsc_ps", bufs=cfg["DVE_SC_BUFS"], space="PSUM"))
        outp_pool = ctx.enter_context(tc.tile_pool(
            name="out_ps", bufs=1, space="PSUM"))
        aexp_pool = ctx.enter_context(tc.tile_pool(
            name="aexp_sb", bufs=cfg["ACT_EXP_BUFS"]))
        dexp_pool = ctx.enter_context(tc.tile_pool(
            name="dexp_sb", bufs=cfg["DVE_EXP_BUFS"]))
        osb_pool = ctx.enter_context(tc.tile_pool(name="o_sb", bufs=2))

        # packs per chunk: 7x [A(2tb), D, D] + 2x [A(2tb)]  (32 tb total)
        packs = [("ADD", 4)] * 7 + [("A", 2)] * 2

        pending = []

        def flush():
            n = len(pending)
            for _ in range(n):
                pending.pop(0)()

        qk_ctr = 0
        state = {"out_ps": None}

        def emit_clear():
            def clear():
                state["out_ps"] = outp_pool.tile([TB, SC], F32, name="out_ps_t")
                nc.tensor.matmul(state["out_ps"][:], lhsT=zeros128[:],
                                 rhs=q_rep[:, 0:SC], start=True, stop=False)
            return clear

        def emit_pv(pieces):
            out_ps = state["out_ps"]

            def pv():
                for expt, col, t in pieces:
                    c = t % 4
                    nc.tensor.matmul(out_ps[32 * c:32 * c + D + 1, :],
                                     lhsT=vT_sb[:, t, :],
                                     rhs=expt[:, ts(col, SC)],
                                     start=False, stop=(t >= NTB - 4),
                                     tile_position=(0, 32 * c))
            return pv

        def emit_fin(s):
            out_ps = state["out_ps"]

            def fin():
                o_sb = osb_pool.tile([TB, SC], F32, name="o_sb")
                nc.vector.tensor_copy(out=o_sb[:], in_=out_ps[:])
                nc.sync.dma_start(out=part[:, ts(s, SC)], in_=o_sb[:])
            return fin

        for s in range(NSC):
            pending.append(emit_clear())
            t0 = 0
            for kind, ntb in packs:
                # allocate tiles for this pack
                a_scp = asc_pool.tile([TB, 2 * SC], F32, name="ascp")
                a_expt = aexp_pool.tile([TB, 2 * SC], F16, name="aexpt")
                d_tiles = []
                if kind == "ADD":
                    for _ in range(2):
                        d_tiles.append((
                            dsc_pool.tile([TB, SC], F32, name="dscp"),
                            dexp_pool.tile([TB, SC], F16, name="dexpt")))
                # QK burst: 4 (or 2) MMs, strips rotate 0..3
                pieces = []
                for j in range(2):
                    t = t0 + j
                    r = qk_ctr % 4
                    qk_ctr += 1
                    nc.tensor.matmul(a_scp[:, ts(j, SC)],
                                     lhsT=k_rep[32 * r:32 * r + 32, ts(t, TB)],
                                     rhs=q_rep[32 * r:32 * r + 32, ts(s, SC)],
                                     start=True, stop=True,
                                     tile_position=(32 * r, 0))
                    pieces.append((a_expt, j, t))
                for i, (d_scp, d_expt) in enumerate(d_tiles):
                    t = t0 + 2 + i
                    r = qk_ctr % 4
                    qk_ctr += 1
                    nc.tensor.matmul(d_scp[:],
                                     lhsT=k_rep[32 * r:32 * r + 32, ts(t, TB)],
                                     rhs=q_rep[32 * r:32 * r + 32, ts(s, SC)],
                                     start=True, stop=True,
                                     tile_position=(32 * r, 0))
                    pieces.append((d_expt, 0, t))
                flush()
                # exps
                nc.scalar.activation(out=a_expt[:],
                                     in_=a_scp[:],
                                     func=mybir.ActivationFunctionType.Exp,
                                     bias=expoff_sb[:], scale=SCALE / REP)
                for d_scp, d_expt in d_tiles:
                    nc.vector.tensor_scalar(out=d_expt[:].bitcast(U16),
                                            in0=d_scp[:],
                                            scalar1=A4, scalar2=B_MAGIC,
                                            op0=mybir.AluOpType.mult,
                                            op1=mybir.AluOpType.add)
                pending.append(emit_pv(pieces))
                t0 += ntb
            pending.append(emit_fin(s))
        flush()


_NC_CACHE = {}


def _build(cfg=None):
    full = dict(DEFAULT_CFG)
    if cfg:
        full.update(cfg)
    key = tuple(sorted(full.items()))
    if key in _NC_CACHE:
        return _NC_CACHE[key]
    nc = bacc.Bacc("TRN2", target_bir_lowering=False, debug=False)
    _emit(nc, cfg=full)
    nc.compile()
    _NC_CACHE[key] = nc
    return nc


def kernel(**inputs):
    x = np.asarray(inputs["x"])
    out_b = np.asarray(inputs["out_b"], dtype=np.float64)
    out_w = np.asarray(inputs["out_w"], dtype=np.float64)
    gn_w = np.asarray(inputs["gn_weight"], dtype=np.float32)
    gn_b = np.asarray(inputs["gn_bias"], dtype=np.float32)
    qkv_w = np.asarray(inputs["qkv_w"], dtype=np.float32)

    x2 = np.ascontiguousarray(np.asarray(x, dtype=np.float32).reshape(C, S))
    gd = np.kron(np.eye(G, dtype=np.float32),
                 np.full((C // G, C // G), float(G) / C, dtype=np.float32))

    in_maps = []
    for h in range(H):
        rq = np.arange(h * D, (h + 1) * D)
        wqk_h = np.ascontiguousarray(
            qkv_w[np.concatenate([rq, C + rq])].T.astype(np.float16))  # [64,16]
        wv_h = np.ascontiguousarray(qkv_w[2 * C + rq].T.astype(np.float16))
        cp = np.zeros((C, CP_W), dtype=np.float32)
        cp[:, 0] = gn_w
        cp[:, 1] = gn_b
        cp[:, 2:66] = gd
        cp[:, 66:74] = wqk_h.view(np.float32)
        cp[:, 74:78] = wv_h.view(np.float32)
        in_maps.append({"x": x2, "cpack": np.ascontiguousarray(cp)})

    nc = _build()
    trace = bool(int(os.environ.get("KERNEL_TRACE", "0")))
    res = run_bass_kernel_spmd(nc, in_maps, core_ids=list(range(H)),
                               trace=trace)
    if trace:
        kernel.last_results = res

    acc = np.zeros((C, S), dtype=np.float64)
    for h, r in enumerate(res.results):
        p = np.asarray(r["part"], dtype=np.float64)
        num = p.reshape(4, 32, S)[:, 0:D + 1, :].sum(axis=0)
        attn = num[0:D] / num[D:D + 1]
        acc += out_w[:, h * D:(h + 1) * D] @ attn
    out = acc + out_b[:, None] + x2.astype(np.float64)
    return out.reshape(x.shape).astype(np.float32)


# revision 17
# speedup vs baseline: 2.5255x; 1.0891x over previous
"""AttentionBlock3D (GroupNorm + 8-head self-attention over 16^3 voxels +
out-projection + residual) on 8 TRN2 NeuronCores — one head per core.

Per-core pipeline:
  - x load (2 halves) + GroupNorm stats (bn_stats + block-diag matmul
    combine), xn = a*x + b stored fp16,
  - q,k via one [64,16] matmul per s-chunk; q/k replicated 16x along
    partitions (fp16, log-depth DMA doubling chains on two queues) so QK^T
    runs as 4 concurrent 32-row PE tiles (tile_position row tiling; the 4x
    replication factor folds into the softmax scale),
  - v^T per t-block with an appended ones column (emits the softmax
    denominator from the same matmul),
  - streaming attention in PACKS of 4 t-blocks (one 2-bank ScalarE tile +
    two 1-bank VectorE tiles): 4-MM QK bursts (row tiles 0..3 concurrent),
    exp split across both engines — ScalarE exact exp (fp16 out) and
    VectorE fast-exp (t = score*A + B -> uint16 round/saturate -> bitcast
    fp16; negatives saturate to 0 = prob 0) — then 4-MM PV bursts
    (col tiles 0..3 concurrent) accumulating into one PSUM bank
    (a zeros-matmul clears has_written each chunk),
  - per chunk: VectorE copies [128,512] PSUM->SBUF, DMA out. No on-device
    division or out-projection.
Host gathers: num_h = sum of 4 col-group rows, attn = num/den,
out = sum_h out_w_h @ attn_h + out_b + x.
"""
import math
import os
from contextlib import ExitStack

import numpy as np

import concourse.bass as bass
import concourse.tile as tile
from concourse import bacc, mybir
from concourse.bass import ts
from concourse.bass_utils import run_bass_kernel_spmd

C, H, G, D = 64, 8, 8, 8
S = 4096
EPS = 1e-5
SCALE = float(D) ** -0.5

SC = 512                # s-chunk (one PSUM bank of fp32)
NSC = S // SC           # 8
TB = 128                # t-block (partition dim of scores^T tiles)
NTB = S // TB           # 32

K2 = -0.5                               # softmax offset, log2 domain
OFF_LN = K2 * math.log(2.0)             # same offset, natural log (ScalarE)
REP = 4                                 # q/k replication factor per strip
A4 = (SCALE / REP) * math.log2(math.e) * 1024.0   # fast-exp multiplier
B_MAGIC = (K2 + 15.0) * 1024.0 - 61.0             # fast-exp bias + mantissa corr

F32 = mybir.dt.float32
F16 = mybir.dt.float16
U16 = mybir.dt.uint16

# packed const layout (fp32 cols): 0 gamma | 1 beta | 2:66 gdiag |
# 66:130 wq_rep (fp16 pairs, [64,128]) | 130:194 wk_rep | 194:198 wv
CP_W = 198

DEFAULT_CFG = {
    "ACT_SC_BUFS": 2,
    "DVE_SC_BUFS": 3,
    "ACT_EXP_BUFS": 4,
    "DVE_EXP_BUFS": 6,
    "DEFER": 2,         # packs of software-pipeline depth for PV emission
}


def _emit(nc, cfg=DEFAULT_CFG):
    x = nc.dram_tensor("x", [C, S], F32, kind="ExternalInput").ap()
    cpack = nc.dram_tensor("cpack", [C, CP_W], F32, kind="ExternalInput").ap()
    part = nc.dram_tensor("part", [TB, S], F32, kind="ExternalOutput").ap()
    with tile.TileContext(nc) as tc:
        _body(nc, tc, x, cpack, part, cfg)


def _body(nc, tc, x, cpack, part, cfg):
    with ExitStack() as ctx:
        const = ctx.enter_context(tc.tile_pool(name="const", bufs=1))
        big = ctx.enter_context(tc.tile_pool(name="big", bufs=1))
        small = ctx.enter_context(tc.tile_pool(name="small", bufs=1))

        # ---- x halves + packed consts ----
        x_sb = big.tile([C, S], F32, name="x_sb")
        nc.sync.dma_start(out=x_sb[:, 0:S // 2], in_=x[:, 0:S // 2])
        nc.sync.dma_start(out=x_sb[:, S // 2:S], in_=x[:, S // 2:S])
        cp_sb = const.tile([C, CP_W], F32, name="cp_sb")
        nc.sync.dma_start(out=cp_sb[:], in_=cpack)
        gamma_sb = cp_sb[:, 0:1]
        beta_sb = cp_sb[:, 1:2]
        gdiag_sb = cp_sb[:, 2:66]
        wq_rep_sb = cp_sb[:, 66:130].bitcast(F16)   # [64, 128] fp16
        wk_rep_sb = cp_sb[:, 130:194].bitcast(F16)  # [64, 128] fp16
        wv_sb = cp_sb[:, 194:198].bitcast(F16)      # [64, 8] fp16

        eps_sb = const.tile([C, 1], F32, name="eps_sb")
        nc.vector.memset(eps_sb[:], EPS)
        zero_sb = const.tile([C, 1], F32, name="zero_sb")
        nc.vector.memset(zero_sb[:], 0.0)
        expoff_sb = const.tile([TB, 1], F32, name="expoff_sb")
        nc.vector.memset(expoff_sb[:], OFF_LN)
        zeros128 = const.tile([TB, TB], F16, name="zeros128")
        nc.vector.memset(zeros128[:], 0.0)

        # ---- GroupNorm stats (per half, 4 bn_stats each) ----
        stats = small.tile([C, NSC, 6], F32, name="stats")
        xv = x_sb[:].rearrange("p (n f) -> p n f", f=SC)
        for j in range(NSC):
            nc.vector.bn_stats(out=stats[:, j, :], in_=xv[:, j, :])
        mv = small.tile([C, 2], F32, name="mv")
        nc.vector.bn_aggr(out=mv[:], in_=stats[:])

        m2 = small.tile([C, 2], F32, name="m2")
        nc.vector.tensor_copy(out=m2[:, 0:1], in_=mv[:, 0:1])
        nc.vector.tensor_mul(out=m2[:, 1:2], in0=mv[:, 0:1], in1=mv[:, 0:1])
        nc.vector.tensor_add(out=m2[:, 1:2], in0=m2[:, 1:2], in1=mv[:, 1:2])

        gst = small.tile([C, 2], F32, name="gst")
        with tc.tile_pool(name="pre_ps", bufs=1, space="PSUM") as pre_ps:
            gst_ps = pre_ps.tile([C, 2], F32, name="gst_ps")
            nc.tensor.matmul(gst_ps[:], lhsT=gdiag_sb, rhs=m2[:],
                             start=True, stop=True)
            nc.vector.tensor_copy(out=gst[:], in_=gst_ps[:])

        var = small.tile([C, 1], F32, name="var")
        nc.vector.tensor_mul(out=var[:], in0=gst[:, 0:1], in1=gst[:, 0:1])
        nc.vector.tensor_sub(out=var[:], in0=gst[:, 1:2], in1=var[:])
        rstd = small.tile([C, 1], F32, name="rstd")
        nc.scalar.activation(out=rstd[:], in_=var[:],
                             func=mybir.ActivationFunctionType.Ln,
                             bias=eps_sb[:], scale=1.0)
        nc.scalar.activation(out=rstd[:], in_=rstd[:],
                             func=mybir.ActivationFunctionType.Exp,
                             bias=zero_sb[:], scale=-0.5)
        a_sc = small.tile([C, 1], F32, name="a_sc")
        nc.vector.tensor_mul(out=a_sc[:], in0=rstd[:], in1=gamma_sb)
        b_sc = small.tile([C, 1], F32, name="b_sc")
        nc.vector.tensor_mul(out=b_sc[:], in0=gst[:, 0:1], in1=a_sc[:])
        nc.vector.tensor_sub(out=b_sc[:], in0=beta_sb, in1=b_sc[:])

        # ---- xn (fp16); q,k replicated 16x straight out of the matmul ----
        xn_sb = big.tile([C, S], F16, name="xn_sb")
        q_rep = big.tile([TB, S], F16, name="q_rep")
        k_rep = big.tile([TB, S], F16, name="k_rep")
        vT_sb = big.tile([TB, NTB, D + 1], F16, name="vT_sb")
        nc.vector.memset(vT_sb[:], 1.0)   # ones col; 0:D overwritten below
        for j in range(NSC):
            nc.vector.tensor_scalar(out=xn_sb[:, ts(j, SC)],
                                    in0=x_sb[:, ts(j, SC)],
                                    scalar1=a_sc[:], scalar2=b_sc[:],
                                    op0=mybir.AluOpType.mult,
                                    op1=mybir.AluOpType.add)
        with tc.tile_pool(name="qk_ps", bufs=2, space="PSUM") as qk_pool, \
             tc.tile_pool(name="vt_ps", bufs=2, space="PSUM") as vt_pool:
            for j in range(NSC):
                q_ps = qk_pool.tile([TB, SC], F32, name="q_ps")
                nc.tensor.matmul(q_ps[:], lhsT=wq_rep_sb,
                                 rhs=xn_sb[:, ts(j, SC)], start=True, stop=True)
                nc.scalar.copy(out=q_rep[:, ts(j, SC)], in_=q_ps[:])
            for j in range(NSC):
                k_ps = qk_pool.tile([TB, SC], F32, name="k_ps", tag="q_ps")
                nc.tensor.matmul(k_ps[:], lhsT=wk_rep_sb,
                                 rhs=xn_sb[:, ts(j, SC)], start=True, stop=True)
                nc.vector.tensor_copy(out=k_rep[:, ts(j, SC)], in_=k_ps[:])
            for j in range(NSC):
                vt_ps = vt_pool.tile([TB, 4, D], F32, name="vt_ps")
                for i in range(4):
                    t = 4 * j + i
                    nc.tensor.matmul(vt_ps[:, i, :],
                                     lhsT=xn_sb[:, ts(t, TB)],
                                     rhs=wv_sb, start=True, stop=True)
                if j % 2 == 0:
                    nc.vector.tensor_copy(
                        out=vT_sb[:, 4 * j:4 * (j + 1), 0:D], in_=vt_ps[:])
                else:
                    nc.scalar.copy(
                        out=vT_sb[:, 4 * j:4 * (j + 1), 0:D], in_=vt_ps[:])

        # ---- attention main loop ----
        asc_pool = ctx.enter_context(tc.tile_pool(
            name="asc_ps", bufs=cfg["ACT_SC_BUFS"], space="PSUM"))
        dsc_pool = ctx.enter_context(tc.tile_pool(
            name="dsc_ps", bufs=cfg["DVE_SC_BUFS"], space="PSUM"))
        outp_pool = ctx.enter_context(tc.tile_pool(
            name="out_ps", bufs=1, space="PSUM"))
        aexp_pool = ctx.enter_context(tc.tile_pool(
            name="aexp_sb", bufs=cfg["ACT_EXP_BUFS"]))
        dexp_pool = ctx.enter_context(tc.tile_pool(
            name="dexp_sb", bufs=cfg["DVE_EXP_BUFS"]))
        osb_pool = ctx.enter_context(tc.tile_pool(name="o_sb", bufs=2))

        # packs per chunk: 7x [A(2tb), D, D] + 2x [A(2tb)]  (32 tb total)
        packs = [("ADD", 4)] * 7 + [("A", 2)] * 2
        defer = cfg["DEFER"]

        pending = []   # (pack_seq, closure), FIFO
        seq = 0

        def flush(min_keep_seq):
            while pending and pending[0][0] <= min_keep_seq:
                pending.pop(0)[1]()

        qk_ctr = 0
        state = {"out_ps": None}

        def emit_clear():
            def clear():
                state["out_ps"] = outp_pool.tile([TB, SC], F32, name="out_ps_t")
                nc.tensor.matmul(state["out_ps"][:], lhsT=zeros128[:],
                                 rhs=q_rep[:, 0:SC], start=True, stop=False)
            return clear

        def emit_pv(pieces):
            def pv():
                out_ps = state["out_ps"]
                for expt, col, t in pieces:
                    c = t % 4
                    nc.tensor.matmul(out_ps[32 * c:32 * c + D + 1, :],
                                     lhsT=vT_sb[:, t, :],
                                     rhs=expt[:, ts(col, SC)],
                                     start=False, stop=(t >= NTB - 4),
                                     tile_position=(0, 32 * c))
            return pv

        def emit_fin(s):
            def fin():
                out_ps = state["out_ps"]
                o_sb = osb_pool.tile([TB, SC], F32, name="o_sb")
                nc.vector.tensor_copy(out=o_sb[:], in_=out_ps[:])
                nc.sync.dma_start(out=part[:, ts(s, SC)], in_=o_sb[:])
            return fin

        for s in range(NSC):
            pending.append((seq, emit_clear()))
            t0 = 0
            for kind, ntb in packs:
                # allocate tiles for this pack
                a_scp = asc_pool.tile([TB, 2 * SC], F32, name="ascp")
                a_expt = aexp_pool.tile([TB, 2 * SC], F16, name="aexpt")
                d_tiles = []
                if kind == "ADD":
                    for _ in range(2):
                        d_tiles.append((
                            dsc_pool.tile([TB, SC], F32, name="dscp"),
                            dexp_pool.tile([TB, SC], F16, name="dexpt")))
                # QK burst: 4 (or 2) MMs, strips rotate 0..3
                pieces = []
                for j in range(2):
                    t = t0 + j
                    r = qk_ctr % 4
                    qk_ctr += 1
                    nc.tensor.matmul(a_scp[:, ts(j, SC)],
                                     lhsT=k_rep[32 * r:32 * r + 32, ts(t, TB)],
                                     rhs=q_rep[32 * r:32 * r + 32, ts(s, SC)],
                                     start=True, stop=True,
                                     tile_position=(32 * r, 0))
                    pieces.append((a_expt, j, t))
                for i, (d_scp, d_expt) in enumerate(d_tiles):
                    t = t0 + 2 + i
                    r = qk_ctr % 4
                    qk_ctr += 1
                    nc.tensor.matmul(d_scp[:],
                                     lhsT=k_rep[32 * r:32 * r + 32, ts(t, TB)],
                                     rhs=q_rep[32 * r:32 * r + 32, ts(s, SC)],
                                     start=True, stop=True,
                                     tile_position=(32 * r, 0))
                    pieces.append((d_expt, 0, t))
                flush(seq - defer)
                # exps
                nc.scalar.activation(out=a_expt[:],
                                     in_=a_scp[:],
                                     func=mybir.ActivationFunctionType.Exp,
                                     bias=expoff_sb[:], scale=SCALE / REP)
                for d_scp, d_expt in d_tiles:
                    nc.vector.tensor_scalar(out=d_expt[:].bitcast(U16),
                                            in0=d_scp[:],
                                            scalar1=A4, scalar2=B_MAGIC,
                                            op0=mybir.AluOpType.mult,
                                            op1=mybir.AluOpType.add)
                pending.append((seq, emit_pv(pieces)))
                t0 += ntb
                seq += 1
            pending.append((seq - 1, emit_fin(s)))
        flush(seq)


_NC_CACHE = {}


def _build(cfg=None):
    full = dict(DEFAULT_CFG)
    if cfg:
        full.update(cfg)
    key = tuple(sorted(full.items()))
    if key in _NC_CACHE:
        return _NC_CACHE[key]
    nc = bacc.Bacc("TRN2", target_bir_lowering=False, debug=False)
    _emit(nc, cfg=full)
    nc.compile()
    _NC_CACHE[key] = nc
    return nc


def kernel(**inputs):
    x = np.asarray(inputs["x"])
    out_b = np.asarray(inputs["out_b"], dtype=np.float64)
    out_w = np.asarray(inputs["out_w"], dtype=np.float64)
    gn_w = np.asarray(inputs["gn_weight"], dtype=np.float32)
    gn_b = np.asarray(inputs["gn_bias"], dtype=np.float32)
    qkv_w = np.asarray(inputs["qkv_w"], dtype=np.float32)

    x2 = np.ascontiguousarray(np.asarray(x, dtype=np.float32).reshape(C, S))
    gd = np.kron(np.eye(G, dtype=np.float32),
                 np.full((C // G, C // G), float(G) / C, dtype=np.float32))

    in_maps = []
    for h in range(H):
        rq = np.arange(h * D, (h + 1) * D)
        wq_rep = np.tile(qkv_w[rq].T, (1, TB // D)).astype(np.float16)
        wk_rep = np.tile(qkv_w[C + rq].T, (1, TB // D)).astype(np.float16)
        wv_h = np.ascontiguousarray(qkv_w[2 * C + rq].T.astype(np.float16))
        cp = np.zeros((C, CP_W), dtype=np.float32)
        cp[:, 0] = gn_w
        cp[:, 1] = gn_b
        cp[:, 2:66] = gd
        cp[:, 66:130] = np.ascontiguousarray(wq_rep).view(np.float32)
        cp[:, 130:194] = np.ascontiguousarray(wk_rep).view(np.float32)
        cp[:, 194:198] = wv_h.view(np.float32)
        in_maps.append({"x": x2, "cpack": np.ascontiguousarray(cp)})

    nc = _build()
    trace = bool(int(os.environ.get("KERNEL_TRACE", "0")))
    res = run_bass_kernel_spmd(nc, in_maps, core_ids=list(range(H)),
                               trace=trace)
    if trace:
        kernel.last_results = res

    acc = np.zeros((C, S), dtype=np.float64)
    for h, r in enumerate(res.results):
        p = np.asarray(r["part"], dtype=np.float64)
        num = p.reshape(4, 32, S)[:, 0:D + 1, :].sum(axis=0)
        attn = num[0:D] / num[D:D + 1]
        acc += out_w[:, h * D:(h + 1) * D] @ attn
    out = acc + out_b[:, None] + x2.astype(np.float64)
    return out.reshape(x.shape).astype(np.float32)


# revision 23
# speedup vs baseline: 2.5297x; 1.0017x over previous
"""AttentionBlock3D (GroupNorm + 8-head self-attention over 16^3 voxels +
out-projection + residual) on 8 TRN2 NeuronCores — one head per core.

Per-core pipeline:
  - x load (2 halves) + GroupNorm stats (bn_stats + block-diag matmul
    combine), xn = a*x + b stored fp16,
  - q,k via one [64,16] matmul per s-chunk; q/k replicated 16x along
    partitions (fp16, log-depth DMA doubling chains on two queues) so QK^T
    runs as 4 concurrent 32-row PE tiles (tile_position row tiling; the 4x
    replication factor folds into the softmax scale),
  - v^T per t-block with an appended ones column (emits the softmax
    denominator from the same matmul),
  - streaming attention in PACKS of 4 t-blocks (one 2-bank ScalarE tile +
    two 1-bank VectorE tiles): 4-MM QK bursts (row tiles 0..3 concurrent),
    exp split across both engines — ScalarE exact exp (fp16 out) and
    VectorE fast-exp (t = score*A + B -> uint16 round/saturate -> bitcast
    fp16; negatives saturate to 0 = prob 0) — then 4-MM PV bursts
    (col tiles 0..3 concurrent) accumulating into one PSUM bank
    (a zeros-matmul clears has_written each chunk),
  - per chunk: VectorE copies [128,512] PSUM->SBUF, DMA out. No on-device
    division or out-projection.
Host gathers: num_h = sum of 4 col-group rows, attn = num/den,
out = sum_h out_w_h @ attn_h + out_b + x.
"""
import math
import os
from contextlib import ExitStack

import numpy as np

import concourse.bass as bass
import concourse.tile as tile
from concourse import bacc, mybir
from concourse.bass import ts
from concourse.bass_utils import run_bass_kernel_spmd

C, H, G, D = 64, 8, 8, 8
S = 4096
EPS = 1e-5
SCALE = float(D) ** -0.5

SC = 512                # s-chunk (one PSUM bank of fp32)
NSC = S // SC           # 8
TB = 128                # t-block (partition dim of scores^T tiles)
NTB = S // TB           # 32

K2 = -0.5                               # softmax offset, log2 domain
OFF_LN = K2 * math.log(2.0)             # same offset, natural log (ScalarE)
REP = 4                                 # q/k replication factor per strip
A4 = (SCALE / REP) * math.log2(math.e) * 1024.0   # fast-exp multiplier
B_MAGIC = (K2 + 15.0) * 1024.0 - 61.0             # fast-exp bias + mantissa corr

F32 = mybir.dt.float32
F16 = mybir.dt.float16
U16 = mybir.dt.uint16

# packed const layout (fp32 cols): 0 gamma | 1 beta | 2:66 gdiag |
# 66:130 wq_rep (fp16 pairs, [64,128]) | 130:194 wk_rep | 194:198 wv
CP_W = 198

DEFAULT_CFG = {
    "ACT_SC_BUFS": 2,
    "DVE_SC_BUFS": 3,
    "ACT_EXP_BUFS": 4,
    "DVE_EXP_BUFS": 6,
    "DEFER": 2,         # packs of software-pipeline depth for PV emission
}


def _emit(nc, cfg=DEFAULT_CFG):
    x = nc.dram_tensor("x", [C, S], F32, kind="ExternalInput").ap()
    cpack = nc.dram_tensor("cpack", [C, CP_W], F32, kind="ExternalInput").ap()
    part = nc.dram_tensor("part", [TB, S], F32, kind="ExternalOutput").ap()
    with tile.TileContext(nc) as tc:
        _body(nc, tc, x, cpack, part, cfg)


def _body(nc, tc, x, cpack, part, cfg):
    with ExitStack() as ctx:
        const = ctx.enter_context(tc.tile_pool(name="const", bufs=1))
        big = ctx.enter_context(tc.tile_pool(name="big", bufs=1))
        small = ctx.enter_context(tc.tile_pool(name="small", bufs=1))

        # ---- x halves + packed consts ----
        x_sb = big.tile([C, S], F32, name="x_sb")
        nc.sync.dma_start(out=x_sb[:, 0:S // 2], in_=x[:, 0:S // 2])
        nc.sync.dma_start(out=x_sb[:, S // 2:S], in_=x[:, S // 2:S])
        cp_sb = const.tile([C, CP_W], F32, name="cp_sb")
        nc.sync.dma_start(out=cp_sb[:], in_=cpack)
        gamma_sb = cp_sb[:, 0:1]
        beta_sb = cp_sb[:, 1:2]
        gdiag_sb = cp_sb[:, 2:66]
        wq_rep_sb = cp_sb[:, 66:130].bitcast(F16)   # [64, 128] fp16
        wk_rep_sb = cp_sb[:, 130:194].bitcast(F16)  # [64, 128] fp16
        wv_sb = cp_sb[:, 194:198].bitcast(F16)      # [64, 8] fp16

        eps_sb = const.tile([C, 1], F32, name="eps_sb")
        nc.vector.memset(eps_sb[:], EPS)
        zero_sb = const.tile([C, 1], F32, name="zero_sb")
        nc.vector.memset(zero_sb[:], 0.0)
        expoff_sb = const.tile([TB, 1], F32, name="expoff_sb")
        nc.vector.memset(expoff_sb[:], OFF_LN)
        zeros128 = const.tile([TB, TB], F16, name="zeros128")
        nc.gpsimd.memset(zeros128[:], 0.0)
        # preload the exp/ln activation table set while x is still loading
        scratch1 = const.tile([C, 1], F32, name="scratch1")
        nc.scalar.activation(out=scratch1[:], in_=zero_sb[:],
                             func=mybir.ActivationFunctionType.Exp,
                             bias=eps_sb[:], scale=1.0)

        # ---- GroupNorm stats (per half, 4 bn_stats each) ----
        stats = small.tile([C, NSC, 6], F32, name="stats")
        xv = x_sb[:].rearrange("p (n f) -> p n f", f=SC)
        for j in range(NSC):
            nc.vector.bn_stats(out=stats[:, j, :], in_=xv[:, j, :])
        mv = small.tile([C, 2], F32, name="mv")
        nc.vector.bn_aggr(out=mv[:], in_=stats[:])

        m2 = small.tile([C, 2], F32, name="m2")
        nc.vector.tensor_copy(out=m2[:, 0:1], in_=mv[:, 0:1])
        nc.vector.tensor_mul(out=m2[:, 1:2], in0=mv[:, 0:1], in1=mv[:, 0:1])
        nc.vector.tensor_add(out=m2[:, 1:2], in0=m2[:, 1:2], in1=mv[:, 1:2])

        gst = small.tile([C, 2], F32, name="gst")
        with tc.tile_pool(name="pre_ps", bufs=1, space="PSUM") as pre_ps:
            gst_ps = pre_ps.tile([C, 2], F32, name="gst_ps")
            nc.tensor.matmul(gst_ps[:], lhsT=gdiag_sb, rhs=m2[:],
                             start=True, stop=True)
            nc.vector.tensor_copy(out=gst[:], in_=gst_ps[:])

        var = small.tile([C, 1], F32, name="var")
        nc.vector.tensor_mul(out=var[:], in0=gst[:, 0:1], in1=gst[:, 0:1])
        nc.vector.tensor_sub(out=var[:], in0=gst[:, 1:2], in1=var[:])
        rstd = small.tile([C, 1], F32, name="rstd")
        nc.scalar.activation(out=rstd[:], in_=var[:],
                             func=mybir.ActivationFunctionType.Ln,
                             bias=eps_sb[:], scale=1.0)
        nc.scalar.activation(out=rstd[:], in_=rstd[:],
                             func=mybir.ActivationFunctionType.Exp,
                             bias=zero_sb[:], scale=-0.5)
        a_sc = small.tile([C, 1], F32, name="a_sc")
        nc.vector.tensor_mul(out=a_sc[:], in0=rstd[:], in1=gamma_sb)
        b_sc = small.tile([C, 1], F32, name="b_sc")
        nc.vector.tensor_mul(out=b_sc[:], in0=gst[:, 0:1], in1=a_sc[:])
        nc.vector.tensor_sub(out=b_sc[:], in0=beta_sb, in1=b_sc[:])

        # ---- xn (fp16, on GPSIMD); q,k replicated 16x out of the matmul ----
        xn_sb = big.tile([C, S], F16, name="xn_sb")
        q_rep = big.tile([TB, S], F16, name="q_rep")
        k_rep = big.tile([TB, S], F16, name="k_rep")
        vT_sb = big.tile([TB, NTB, D + 1], F16, name="vT_sb")
        nc.gpsimd.memset(vT_sb[:], 1.0)   # ones col; 0:D overwritten below
        for j in range(NSC):
            nc.gpsimd.tensor_scalar(out=xn_sb[:, ts(j, SC)],
                                    in0=x_sb[:, ts(j, SC)],
                                    scalar1=a_sc[:], scalar2=b_sc[:],
                                    op0=mybir.AluOpType.mult,
                                    op1=mybir.AluOpType.add)
        with tc.tile_pool(name="qk_ps", bufs=2, space="PSUM") as qk_pool, \
             tc.tile_pool(name="vt_ps", bufs=2, space="PSUM") as vt_pool:
            for jj in range(NSC // 2):
                q_ps = qk_pool.tile([TB, 2, SC], F32, name="q_ps")
                for i in range(2):
                    nc.tensor.matmul(q_ps[:, i, :], lhsT=wq_rep_sb,
                                     rhs=xn_sb[:, ts(2 * jj + i, SC)],
                                     start=True, stop=True)
                nc.scalar.copy(
                    out=q_rep[:, ts(jj, 2 * SC)].rearrange(
                        "p (a b) -> p a b", a=2),
                    in_=q_ps[:])
            for jj in range(NSC // 2):
                k_ps = qk_pool.tile([TB, 2, SC], F32, name="k_ps", tag="q_ps")
                for i in range(2):
                    nc.tensor.matmul(k_ps[:, i, :], lhsT=wk_rep_sb,
                                     rhs=xn_sb[:, ts(2 * jj + i, SC)],
                                     start=True, stop=True)
                nc.vector.tensor_copy(
                    out=k_rep[:, ts(jj, 2 * SC)].rearrange(
                        "p (a b) -> p a b", a=2),
                    in_=k_ps[:])
            for j in range(NSC):
                vt_ps = vt_pool.tile([TB, 4, D], F32, name="vt_ps")
                for i in range(4):
                    t = 4 * j + i
                    nc.tensor.matmul(vt_ps[:, i, :],
                                     lhsT=xn_sb[:, ts(t, TB)],
                                     rhs=wv_sb, start=True, stop=True)
                if j % 2 == 0:
                    nc.vector.tensor_copy(
                        out=vT_sb[:, 4 * j:4 * (j + 1), 0:D], in_=vt_ps[:])
                else:
                    nc.scalar.copy(
                        out=vT_sb[:, 4 * j:4 * (j + 1), 0:D], in_=vt_ps[:])

        # ---- attention main loop ----
        asc_pool = ctx.enter_context(tc.tile_pool(
            name="asc_ps", bufs=cfg["ACT_SC_BUFS"], space="PSUM"))
        dsc_pool = ctx.enter_context(tc.tile_pool(
            name="dsc_ps", bufs=cfg["DVE_SC_BUFS"], space="PSUM"))
        outp_pool = ctx.enter_context(tc.tile_pool(
            name="out_ps", bufs=1, space="PSUM"))
        aexp_pool = ctx.enter_context(tc.tile_pool(
            name="aexp_sb", bufs=cfg["ACT_EXP_BUFS"]))
        dexp_pool = ctx.enter_context(tc.tile_pool(
            name="dexp_sb", bufs=cfg["DVE_EXP_BUFS"]))
        osb_pool = ctx.enter_context(tc.tile_pool(name="o_sb", bufs=2))

        # packs of 4 t-blocks; even chunks 8x[ADD] (16 ACT tb),
        # odd chunks 7x[ADD]+[AA] (18 ACT tb) — balances ACT vs DVE exp load
        defer = cfg["DEFER"]

        pending = []   # (pack_seq, closure), FIFO
        seq = 0

        def flush(min_keep_seq):
            while pending and pending[0][0] <= min_keep_seq:
                pending.pop(0)[1]()

        qk_ctr = 0
        state = {"out_ps": None}

        def emit_clear():
            def clear():
                state["out_ps"] = outp_pool.tile([TB, SC], F32, name="out_ps_t")
                nc.tensor.matmul(state["out_ps"][:], lhsT=zeros128[:],
                                 rhs=q_rep[:, 0:SC], start=True, stop=False)
            return clear

        def emit_pv(pieces):
            def pv():
                out_ps = state["out_ps"]
                for expt, col, t in pieces:
                    c = t % 4
                    nc.tensor.matmul(out_ps[32 * c:32 * c + D + 1, :],
                                     lhsT=vT_sb[:, t, :],
                                     rhs=expt[:, ts(col, SC)],
                                     start=False, stop=(t >= NTB - 4),
                                     tile_position=(0, 32 * c))
            return pv

        def emit_fin(s):
            def fin():
                out_ps = state["out_ps"]
                o_sb = osb_pool.tile([TB, SC], F32, name="o_sb")
                if s % 2 == 0:
                    nc.scalar.copy(out=o_sb[:], in_=out_ps[:])
                else:
                    nc.vector.tensor_copy(out=o_sb[:], in_=out_ps[:])
                nc.sync.dma_start(out=part[:, ts(s, SC)], in_=o_sb[:])
            return fin

        def qk_mm(dst_ap, t, s):
            nonlocal qk_ctr
            r = qk_ctr % 4
            qk_ctr += 1
            nc.tensor.matmul(dst_ap,
                             lhsT=k_rep[32 * r:32 * r + 32, ts(t, TB)],
                             rhs=q_rep[32 * r:32 * r + 32, ts(s, SC)],
                             start=True, stop=True,
                             tile_position=(32 * r, 0))

        for s in range(NSC):
            packs = ["ADD"] * 8 if s % 2 == 0 else ["ADD"] * 7 + ["AA"]
            pending.append((seq, emit_clear()))
            t0 = 0
            for kind in packs:
                a_tiles = []
                d_tiles = []
                if kind == "ADD":
                    a_tiles.append((
                        asc_pool.tile([TB, 2 * SC], F32, name="ascp"),
                        aexp_pool.tile([TB, 2 * SC], F16, name="aexpt")))
                    for _ in range(2):
                        d_tiles.append((
                            dsc_pool.tile([TB, SC], F32, name="dscp"),
                            dexp_pool.tile([TB, SC], F16, name="dexpt")))
                else:  # AA
                    for _ in range(2):
                        a_tiles.append((
                            asc_pool.tile([TB, 2 * SC], F32, name="ascp"),
                            aexp_pool.tile([TB, 2 * SC], F16, name="aexpt")))
                # QK burst: 4 MMs, strips rotate 0..3
                pieces = []
                t = t0
                for a_scp, a_expt in a_tiles:
                    for j in range(2):
                        qk_mm(a_scp[:, ts(j, SC)], t, s)
                        pieces.append((a_expt, j, t))
                        t += 1
                for d_scp, d_expt in d_tiles:
                    qk_mm(d_scp[:], t, s)
                    pieces.append((d_expt, 0, t))
                    t += 1
                flush(seq - defer)
                # exps
                for a_scp, a_expt in a_tiles:
                    nc.scalar.activation(out=a_expt[:],
                                         in_=a_scp[:],
                                         func=mybir.ActivationFunctionType.Exp,
                                         bias=expoff_sb[:], scale=SCALE / REP)
                for d_scp, d_expt in d_tiles:
                    nc.vector.tensor_scalar(out=d_expt[:].bitcast(U16),
                                            in0=d_scp[:],
                                            scalar1=A4, scalar2=B_MAGIC,
                                            op0=mybir.AluOpType.mult,
                                            op1=mybir.AluOpType.add)
                pending.append((seq, emit_pv(pieces)))
                t0 += 4
                seq += 1
            pending.append((seq - 1, emit_fin(s)))
        flush(seq)


_NC_CACHE = {}


def _build(cfg=None):
    full = dict(DEFAULT_CFG)
    if cfg:
        full.update(cfg)
    key = tuple(sorted(full.items()))
    if key in _NC_CACHE:
        return _NC_CACHE[key]
    nc = bacc.Bacc("TRN2", target_bir_lowering=False, debug=False)
    _emit(nc, cfg=full)
    nc.compile()
    _NC_CACHE[key] = nc
    return nc


def kernel(**inputs):
    x = np.asarray(inputs["x"])
    out_b = np.asarray(inputs["out_b"], dtype=np.float64)
    out_w = np.asarray(inputs["out_w"], dtype=np.float64)
    gn_w = np.asarray(inputs["gn_weight"], dtype=np.float32)
    gn_b = np.asarray(inputs["gn_bias"], dtype=np.float32)
    qkv_w = np.asarray(inputs["qkv_w"], dtype=np.float32)

    x2 = np.ascontiguousarray(np.asarray(x, dtype=np.float32).reshape(C, S))
    gd = np.kron(np.eye(G, dtype=np.float32),
                 np.full((C // G, C // G), float(G) / C, dtype=np.float32))

    in_maps = []
    for h in range(H):
        rq = np.arange(h * D, (h + 1) * D)
        wq_rep = np.tile(qkv_w[rq].T, (1, TB // D)).astype(np.float16)
        wk_rep = np.tile(qkv_w[C + rq].T, (1, TB // D)).astype(np.float16)
        wv_h = np.ascontiguousarray(qkv_w[2 * C + rq].T.astype(np.float16))
        cp = np.zeros((C, CP_W), dtype=np.float32)
        cp[:, 0] = gn_w
        cp[:, 1] = gn_b
        cp[:, 2:66] = gd
        cp[:, 66:130] = np.ascontiguousarray(wq_rep).view(np.float32)
        cp[:, 130:194] = np.ascontiguousarray(wk_rep).view(np.float32)
        cp[:, 194:198] = wv_h.view(np.float32)
        in_maps.append({"x": x2, "cpack": np.ascontiguousarray(cp)})

    nc = _build()
    trace = bool(int(os.environ.get("KERNEL_TRACE", "0")))
    res = run_bass_kernel_spmd(nc, in_maps, core_ids=list(range(H)),
                               trace=trace)
    if trace:
        kernel.last_results = res

    acc = np.zeros((C, S), dtype=np.float64)
    for h, r in enumerate(res.results):
        p = np.asarray(r["part"], dtype=np.float64)
        num = p.reshape(4, 32, S)[:, 0:D + 1, :].sum(axis=0)
        attn = num[0:D] / num[D:D + 1]
        acc += out_w[:, h * D:(h + 1) * D] @ attn
    out = acc + out_b[:, None] + x2.astype(np.float64)
    return out.reshape(x.shape).astype(np.float32)


# revision 27
# speedup vs baseline: 2.5745x; 1.0177x over previous
"""AttentionBlock3D (GroupNorm + 8-head self-attention over 16^3 voxels +
out-projection + residual) on 8 TRN2 NeuronCores — one head per core.

Per-core pipeline:
  - x load (2 halves) + GroupNorm stats (bn_stats + block-diag matmul
    combine), xn = a*x + b stored fp16,
  - q,k via one [64,16] matmul per s-chunk; q/k replicated 16x along
    partitions (fp16, log-depth DMA doubling chains on two queues) so QK^T
    runs as 4 concurrent 32-row PE tiles (tile_position row tiling; the 4x
    replication factor folds into the softmax scale),
  - v^T per t-block with an appended ones column (emits the softmax
    denominator from the same matmul),
  - streaming attention in PACKS of 4 t-blocks (one 2-bank ScalarE tile +
    two 1-bank VectorE tiles): 4-MM QK bursts (row tiles 0..3 concurrent),
    exp split across both engines — ScalarE exact exp (fp16 out) and
    VectorE fast-exp (t = score*A + B -> uint16 round/saturate -> bitcast
    fp16; negatives saturate to 0 = prob 0) — then 4-MM PV bursts
    (col tiles 0..3 concurrent) accumulating into one PSUM bank
    (a zeros-matmul clears has_written each chunk),
  - per chunk: VectorE copies [128,512] PSUM->SBUF, DMA out. No on-device
    division or out-projection.
Host gathers: num_h = sum of 4 col-group rows, attn = num/den,
out = sum_h out_w_h @ attn_h + out_b + x.
"""
import math
import os
from contextlib import ExitStack

import numpy as np

import concourse.bass as bass
import concourse.tile as tile
from concourse import bacc, mybir
from concourse.bass import ts
from concourse.bass_utils import run_bass_kernel_spmd

C, H, G, D = 64, 8, 8, 8
S = 4096
EPS = 1e-5
SCALE = float(D) ** -0.5

SC = 512                # s-chunk (one PSUM bank of fp32)
NSC = S // SC           # 8
TB = 128                # t-block (partition dim of scores^T tiles)
NTB = S // TB           # 32

K2 = -0.5                               # softmax offset, log2 domain
OFF_LN = K2 * math.log(2.0)             # same offset, natural log (ScalarE)
REP = 4                                 # q/k replication factor per strip
A4 = (SCALE / REP) * math.log2(math.e) * 1024.0   # fast-exp multiplier
B_MAGIC = (K2 + 15.0) * 1024.0 - 61.0             # fast-exp bias + mantissa corr

F32 = mybir.dt.float32
F16 = mybir.dt.float16
U16 = mybir.dt.uint16

# packed const layout (fp32 cols): 0 gamma | 1 beta | 2:66 gdiag |
# 66:130 wq_rep (fp16 pairs, [64,128]) | 130:194 wk_rep | 194:198 wv
CP_W = 198

DEFAULT_CFG = {
    "ACT_SC_BUFS": 2,
    "DVE_SC_BUFS": 3,
    "ACT_EXP_BUFS": 5,
    "DVE_EXP_BUFS": 7,
    "DEFER": 3,         # packs of software-pipeline depth for PV emission
}


def _emit(nc, cfg=DEFAULT_CFG):
    x = nc.dram_tensor("x", [C, S], F32, kind="ExternalInput").ap()
    cpack = nc.dram_tensor("cpack", [C, CP_W], F32, kind="ExternalInput").ap()
    part = nc.dram_tensor("part", [TB, S], F32, kind="ExternalOutput").ap()
    with tile.TileContext(nc) as tc:
        _body(nc, tc, x, cpack, part, cfg)


def _body(nc, tc, x, cpack, part, cfg):
    with ExitStack() as ctx:
        const = ctx.enter_context(tc.tile_pool(name="const", bufs=1))
        big = ctx.enter_context(tc.tile_pool(name="big", bufs=1))
        small = ctx.enter_context(tc.tile_pool(name="small", bufs=1))

        # ---- x chunks (two issue queues) + packed consts ----
        x_sb = big.tile([C, S], F32, name="x_sb")
        for j in range(NSC):
            eng = nc.sync if j % 2 == 0 else nc.scalar
            eng.dma_start(out=x_sb[:, ts(j, SC)], in_=x[:, ts(j, SC)])
        cp_sb = const.tile([C, CP_W], F32, name="cp_sb")
        nc.sync.dma_start(out=cp_sb[:], in_=cpack)
        gamma_sb = cp_sb[:, 0:1]
        beta_sb = cp_sb[:, 1:2]
        gdiag_sb = cp_sb[:, 2:66]
        wq_rep_sb = cp_sb[:, 66:130].bitcast(F16)   # [64, 128] fp16
        wk_rep_sb = cp_sb[:, 130:194].bitcast(F16)  # [64, 128] fp16
        wv_sb = cp_sb[:, 194:198].bitcast(F16)      # [64, 8] fp16

        eps_sb = const.tile([C, 1], F32, name="eps_sb")
        nc.vector.memset(eps_sb[:], EPS)
        zero_sb = const.tile([C, 1], F32, name="zero_sb")
        nc.vector.memset(zero_sb[:], 0.0)
        expoff_sb = const.tile([TB, 1], F32, name="expoff_sb")
        nc.vector.memset(expoff_sb[:], OFF_LN)
        zeros128 = const.tile([TB, TB], F16, name="zeros128")
        nc.gpsimd.memset(zeros128[:], 0.0)
        # preload the natural_log_exp table set (covers Ln AND Exp) while x
        # is still loading, so no table load lands on the critical path
        scratch1 = const.tile([C, 1], F32, name="scratch1")
        nc.scalar.activation(out=scratch1[:], in_=zero_sb[:],
                             func=mybir.ActivationFunctionType.Ln,
                             bias=eps_sb[:], scale=1.0)

        # ---- GroupNorm stats (per half, 4 bn_stats each) ----
        stats = small.tile([C, NSC, 6], F32, name="stats")
        xv = x_sb[:].rearrange("p (n f) -> p n f", f=SC)
        for j in range(NSC):
            nc.vector.bn_stats(out=stats[:, j, :], in_=xv[:, j, :])
        mv = small.tile([C, 2], F32, name="mv")
        nc.vector.bn_aggr(out=mv[:], in_=stats[:])

        m2 = small.tile([C, 2], F32, name="m2")
        nc.vector.tensor_copy(out=m2[:, 0:1], in_=mv[:, 0:1])
        nc.vector.tensor_mul(out=m2[:, 1:2], in0=mv[:, 0:1], in1=mv[:, 0:1])
        nc.vector.tensor_add(out=m2[:, 1:2], in0=m2[:, 1:2], in1=mv[:, 1:2])

        gst = small.tile([C, 2], F32, name="gst")
        with tc.tile_pool(name="pre_ps", bufs=1, space="PSUM") as pre_ps:
            gst_ps = pre_ps.tile([C, 2], F32, name="gst_ps")
            nc.tensor.matmul(gst_ps[:], lhsT=gdiag_sb, rhs=m2[:],
                             start=True, stop=True)
            nc.vector.tensor_copy(out=gst[:], in_=gst_ps[:])

        var = small.tile([C, 1], F32, name="var")
        nc.vector.tensor_mul(out=var[:], in0=gst[:, 0:1], in1=gst[:, 0:1])
        nc.vector.tensor_sub(out=var[:], in0=gst[:, 1:2], in1=var[:])
        rstd = small.tile([C, 1], F32, name="rstd")
        nc.scalar.activation(out=rstd[:], in_=var[:],
                             func=mybir.ActivationFunctionType.Ln,
                             bias=eps_sb[:], scale=1.0)
        nc.scalar.activation(out=rstd[:], in_=rstd[:],
                             func=mybir.ActivationFunctionType.Exp,
                             bias=zero_sb[:], scale=-0.5)
        a_sc = small.tile([C, 1], F32, name="a_sc")
        nc.vector.tensor_mul(out=a_sc[:], in0=rstd[:], in1=gamma_sb)
        b_sc = small.tile([C, 1], F32, name="b_sc")
        nc.vector.tensor_mul(out=b_sc[:], in0=gst[:, 0:1], in1=a_sc[:])
        nc.vector.tensor_sub(out=b_sc[:], in0=beta_sb, in1=b_sc[:])

        # ---- xn (fp16, on GPSIMD); q,k replicated 16x out of the matmul ----
        xn_sb = big.tile([C, S], F16, name="xn_sb")
        q_rep = big.tile([TB, S], F16, name="q_rep")
        k_rep = big.tile([TB, S], F16, name="k_rep")
        vT_sb = big.tile([TB, NTB, D + 1], F16, name="vT_sb")
        nc.gpsimd.memset(vT_sb[:], 1.0)   # ones col; 0:D overwritten below
        for j in range(NSC):
            nc.gpsimd.tensor_scalar(out=xn_sb[:, ts(j, SC)],
                                    in0=x_sb[:, ts(j, SC)],
                                    scalar1=a_sc[:], scalar2=b_sc[:],
                                    op0=mybir.AluOpType.mult,
                                    op1=mybir.AluOpType.add)
        with tc.tile_pool(name="qk_ps", bufs=2, space="PSUM") as qk_pool, \
             tc.tile_pool(name="vt_ps", bufs=2, space="PSUM") as vt_pool:
            for jj in range(NSC // 2):
                q_ps = qk_pool.tile([TB, 2, SC], F32, name="q_ps")
                for i in range(2):
                    nc.tensor.matmul(q_ps[:, i, :], lhsT=wq_rep_sb,
                                     rhs=xn_sb[:, ts(2 * jj + i, SC)],
                                     start=True, stop=True)
                nc.scalar.copy(
                    out=q_rep[:, ts(jj, 2 * SC)].rearrange(
                        "p (a b) -> p a b", a=2),
                    in_=q_ps[:])
            for jj in range(NSC // 2):
                k_ps = qk_pool.tile([TB, 2, SC], F32, name="k_ps", tag="q_ps")
                for i in range(2):
                    nc.tensor.matmul(k_ps[:, i, :], lhsT=wk_rep_sb,
                                     rhs=xn_sb[:, ts(2 * jj + i, SC)],
                                     start=True, stop=True)
                nc.vector.tensor_copy(
                    out=k_rep[:, ts(jj, 2 * SC)].rearrange(
                        "p (a b) -> p a b", a=2),
                    in_=k_ps[:])
            for j in range(NSC):
                vt_ps = vt_pool.tile([TB, 4, D], F32, name="vt_ps")
                for i in range(4):
                    t = 4 * j + i
                    nc.tensor.matmul(vt_ps[:, i, :],
                                     lhsT=xn_sb[:, ts(t, TB)],
                                     rhs=wv_sb, start=True, stop=True)
                if j % 2 == 0:
                    nc.vector.tensor_copy(
                        out=vT_sb[:, 4 * j:4 * (j + 1), 0:D], in_=vt_ps[:])
                else:
                    nc.scalar.copy(
                        out=vT_sb[:, 4 * j:4 * (j + 1), 0:D], in_=vt_ps[:])

        # ---- attention main loop ----
        asc_pool = ctx.enter_context(tc.tile_pool(
            name="asc_ps", bufs=cfg["ACT_SC_BUFS"], space="PSUM"))
        dsc_pool = ctx.enter_context(tc.tile_pool(
            name="dsc_ps", bufs=cfg["DVE_SC_BUFS"], space="PSUM"))
        outp_pool = ctx.enter_context(tc.tile_pool(
            name="out_ps", bufs=1, space="PSUM"))
        aexp_pool = ctx.enter_context(tc.tile_pool(
            name="aexp_sb", bufs=cfg["ACT_EXP_BUFS"]))
        dexp_pool = ctx.enter_context(tc.tile_pool(
            name="dexp_sb", bufs=cfg["DVE_EXP_BUFS"]))
        osb_pool = ctx.enter_context(tc.tile_pool(name="o_sb", bufs=2))

        # packs of 4 t-blocks; even chunks 8x[ADD] (16 ACT tb),
        # odd chunks 7x[ADD]+[AA] (18 ACT tb) — balances ACT vs DVE exp load
        defer = cfg["DEFER"]

        pending = []   # (pack_seq, closure), FIFO
        seq = 0

        def flush(min_keep_seq):
            while pending and pending[0][0] <= min_keep_seq:
                pending.pop(0)[1]()

        qk_ctr = 0
        state = {"out_ps": None}

        def emit_clear():
            def clear():
                state["out_ps"] = outp_pool.tile([TB, SC], F32, name="out_ps_t")
                nc.tensor.matmul(state["out_ps"][:], lhsT=zeros128[:],
                                 rhs=q_rep[:, 0:SC], start=True, stop=False)
            return clear

        def emit_pv(pieces):
            def pv():
                out_ps = state["out_ps"]
                for expt, col, t in pieces:
                    c = t % 4
                    nc.tensor.matmul(out_ps[32 * c:32 * c + D + 1, :],
                                     lhsT=vT_sb[:, t, :],
                                     rhs=expt[:, ts(col, SC)],
                                     start=False, stop=(t >= NTB - 4),
                                     tile_position=(0, 32 * c))
            return pv

        def emit_fin(s):
            def fin():
                out_ps = state["out_ps"]
                o_sb = osb_pool.tile([TB, SC], F32, name="o_sb")
                if s % 2 == 0:
                    nc.scalar.copy(out=o_sb[:], in_=out_ps[:])
                else:
                    nc.vector.tensor_copy(out=o_sb[:], in_=out_ps[:])
                nc.sync.dma_start(out=part[:, ts(s, SC)], in_=o_sb[:])
            return fin

        def qk_mm(dst_ap, t, s):
            nonlocal qk_ctr
            r = qk_ctr % 4
            qk_ctr += 1
            nc.tensor.matmul(dst_ap,
                             lhsT=k_rep[32 * r:32 * r + 32, ts(t, TB)],
                             rhs=q_rep[32 * r:32 * r + 32, ts(s, SC)],
                             start=True, stop=True,
                             tile_position=(32 * r, 0))

        for s in range(NSC):
            packs = (["ADD"] * 8 if s % 2 == 0 else
                     ["ADD"] * 3 + ["AA"] + ["ADD"] * 4)
            pending.append((seq, emit_clear()))
            t0 = 0
            for kind in packs:
                a_tiles = []
                d_tiles = []
                if kind == "ADD":
                    a_tiles.append((
                        asc_pool.tile([TB, 2 * SC], F32, name="ascp"),
                        aexp_pool.tile([TB, 2 * SC], F16, name="aexpt")))
                    for _ in range(2):
                        d_tiles.append((
                            dsc_pool.tile([TB, SC], F32, name="dscp"),
                            dexp_pool.tile([TB, SC], F16, name="dexpt")))
                else:  # AA
                    for _ in range(2):
                        a_tiles.append((
                            asc_pool.tile([TB, 2 * SC], F32, name="ascp"),
                            aexp_pool.tile([TB, 2 * SC], F16, name="aexpt")))
                # QK burst: 4 MMs, strips rotate 0..3
                pieces = []
                t = t0
                for a_scp, a_expt in a_tiles:
                    for j in range(2):
                        qk_mm(a_scp[:, ts(j, SC)], t, s)
                        pieces.append((a_expt, j, t))
                        t += 1
                for d_scp, d_expt in d_tiles:
                    qk_mm(d_scp[:], t, s)
                    pieces.append((d_expt, 0, t))
                    t += 1
                flush(seq - defer)
                # exps
                for a_scp, a_expt in a_tiles:
                    nc.scalar.activation(out=a_expt[:],
                                         in_=a_scp[:],
                                         func=mybir.ActivationFunctionType.Exp,
                                         bias=expoff_sb[:], scale=SCALE / REP)
                for d_scp, d_expt in d_tiles:
                    nc.vector.tensor_scalar(out=d_expt[:].bitcast(U16),
                                            in0=d_scp[:],
                                            scalar1=A4, scalar2=B_MAGIC,
                                            op0=mybir.AluOpType.mult,
                                            op1=mybir.AluOpType.add)
                pending.append((seq, emit_pv(pieces)))
                t0 += 4
                seq += 1
            pending.append((seq - 1, emit_fin(s)))
        flush(seq)


_NC_CACHE = {}


def _build(cfg=None):
    full = dict(DEFAULT_CFG)
    if cfg:
        full.update(cfg)
    key = tuple(sorted(full.items()))
    if key in _NC_CACHE:
        return _NC_CACHE[key]
    nc = bacc.Bacc("TRN2", target_bir_lowering=False, debug=False)
    _emit(nc, cfg=full)
    nc.compile()
    _NC_CACHE[key] = nc
    return nc


def kernel(**inputs):
    x = np.asarray(inputs["x"])
    out_b = np.asarray(inputs["out_b"], dtype=np.float64)
    out_w = np.asarray(inputs["out_w"], dtype=np.float64)
    gn_w = np.asarray(inputs["gn_weight"], dtype=np.float32)
    gn_b = np.asarray(inputs["gn_bias"], dtype=np.float32)
    qkv_w = np.asarray(inputs["qkv_w"], dtype=np.float32)

    x2 = np.ascontiguousarray(np.asarray(x, dtype=np.float32).reshape(C, S))
    gd = np.kron(np.eye(G, dtype=np.float32),
                 np.full((C // G, C // G), float(G) / C, dtype=np.float32))

    in_maps = []
    for h in range(H):
        rq = np.arange(h * D, (h + 1) * D)
        wq_rep = np.tile(qkv_w[rq].T, (1, TB // D)).astype(np.float16)
        wk_rep = np.tile(qkv_w[C + rq].T, (1, TB // D)).astype(np.float16)
        wv_h = np.ascontiguousarray(qkv_w[2 * C + rq].T.astype(np.float16))
        cp = np.zeros((C, CP_W), dtype=np.float32)
        cp[:, 0] = gn_w
        cp[:, 1] = gn_b
        cp[:, 2:66] = gd
        cp[:, 66:130] = np.ascontiguousarray(wq_rep).view(np.float32)
        cp[:, 130:194] = np.ascontiguousarray(wk_rep).view(np.float32)
        cp[:, 194:198] = wv_h.view(np.float32)
        in_maps.append({"x": x2, "cpack": np.ascontiguousarray(cp)})

    nc = _build()
    trace = bool(int(os.environ.get("KERNEL_TRACE", "0")))
    res = run_bass_kernel_spmd(nc, in_maps, core_ids=list(range(H)),
                               trace=trace)
    if trace:
        kernel.last_results = res

    acc = np.zeros((C, S), dtype=np.float64)
    for h, r in enumerate(res.results):
        p = np.asarray(r["part"], dtype=np.float64)
        num = p.reshape(4, 32, S)[:, 0:D + 1, :].sum(axis=0)
        attn = num[0:D] / num[D:D + 1]
        acc += out_w[:, h * D:(h + 1) * D] @ attn
    out = acc + out_b[:, None] + x2.astype(np.float64)
    return out.reshape(x.shape).astype(np.float32)
